# revision 1
# baseline (speedup 1.0000x reference)
"""Equivariant attention (gnn_message_passing) on 8 Trainium2 NeuronCores.

Strategy (head-sharded tensor parallel, core c owns head c):

The reference materializes [H, N, E] scores/attn over E=8192 edges. Here the
edge dimension is collapsed onto the N=512 atoms at projection level:

  scores[h, n, e]   = sf[h, n, a_e] + bias[h, edge_map[e]]     (a_e = atom_index)
  attn-softmax per (batch-segment, n) then  out = attn @ vh_edges

factors exactly into dense [N, N] algebra with two tiny per-(segment, atom)
tables (NSEG=16 x N=512):

  D[g, m] = sum_{e in seg g, a_e = m} env_e   * exp(b_e)
  C[g, m] = sum_{e in seg g, a_e = m} env_e^2 * exp(b_e)
  den[g, n]  = sum_m exp(sf[m, n]) * D[g, m]           (one matmul)
  Aagg[m, n] = exp(sf[m, n]) * sum_g C[g, m] / den[g, n]
  out[n, f]  = Aagg^T @ vh[m, f]                        (one matmul)

The running-max subtraction in the reference softmax cancels exactly (up to a
+1e-16 epsilon whose relative effect is ~1e-16) and |scale*sf + b| < 20, so
unnormalized exp is safe in f32.

D/C are built on-device from "slot tensors": host packs per-(atom, segment)
edge lists into a fixed-width [128, 4*16*L2] layout (env & bias values; pads
have env=0 so they vanish), and a single free-axis reduce per table produces
it. Only integer index bookkeeping and value re-layout happen on host.

Output stage: per-head attn-out is written into a zero-padded [N, S, H, d]
channel-striped DRAM buffer via an indirect row-scatter (row ids are a
per-core input, so one NEFF serves all cores), then one ReduceScatter(add)
hands each core its 64-atom slice with all 256 channels assembled. LayerNorm
+ output projection run on that slice; the host concatenates the 8 slices.
"""

import os
import numpy as np

import concourse.bass as bass
import concourse.tile as tile
from concourse import mybir
from concourse.bass_utils import run_bass_kernel_spmd
from concourse.masks import make_identity

# ---------------------------------------------------------------- constants
H, LMAX, NSEG = 8, 2, 16
S = (LMAX + 1) ** 2          # 9 spherical components
N, E, CIN, CH = 512, 8192, 128, 256
D = CH // H                  # 32 per-head channels
F = S * D                    # 288 per-head feature width
NT = N // 128                # 4 atom tiles
NR = N // H                  # 64 atoms per core in the LN/out stage
EPS = 1e-7
SCALE = float(np.sqrt(D / 3.0) / D)
L_OF_M = np.floor(np.sqrt(np.arange(S))).astype(np.int64)
F32 = mybir.dt.float32
F32R = mybir.dt.float32r
AF = mybir.ActivationFunctionType
ALU = mybir.AluOpType

_DBG = bool(int(os.environ.get("KBDBG", "0")))


def _split_multiwaits(nc: bass.Bass, limit: int = 1):
    """This walrus build rejects instructions carrying more than one semaphore
    wait (and Drains carrying any). Hoist excess waits onto NOPs inserted just
    before the instruction on the same engine - semantically identical."""
    for f in nc.m.functions:
        for blk in f.blocks:
            changed = False
            out = []
            for inst in blk.instructions:
                si = inst.sync_info
                waits = list(si.on_wait) if si is not None else []
                keep = 0 if inst.opcode == "Drain" else limit
                if len(waits) > keep:
                    hoist = waits[: len(waits) - keep]
                    rest = waits[len(waits) - keep:]
                    for w in hoist:
                        nop = mybir.InstNoOp(
                            name=f"{inst.name}-w{len(out)}", ins=[], outs=[]
                        )
                        nop.engine = inst.engine
                        nop.sync_info = mybir.SyncInfo(on_wait=[w], on_update=[])
                        out.append(nop)
                    inst.sync_info = mybir.SyncInfo(
                        on_wait=rest, on_update=list(si.on_update)
                    )
                    changed = True
                out.append(inst)
            if changed:
                blk.instructions = out


def build_bass(L2: int, loop_R: int | None = None) -> bass.Bass:
    """One SPMD program; per-core data (weight slices, bias slots, scatter row
    ids) comes in as inputs. L2 = slot width per (atom, segment) cell."""
    W = NT * NSEG * L2  # slot tensor free width per partition

    nc = bass.Bass("TRN2", target_bir_lowering=False, debug=False, num_devices=H)

    # ------------------------------------------------------------- tensors
    q_d = nc.dram_tensor("q", [N, S * CIN], F32, kind="ExternalInput")
    k_d = nc.dram_tensor("k", [N, S * CIN], F32, kind="ExternalInput")
    v_d = nc.dram_tensor("v", [N, S * CIN], F32, kind="ExternalInput")
    wq_d = nc.dram_tensor("wq", [S * CIN, D], F32, kind="ExternalInput")  # (s,i),o
    wk_d = nc.dram_tensor("wk", [S * CIN, D], F32, kind="ExternalInput")
    wv_d = nc.dram_tensor("wv", [S * CIN, D], F32, kind="ExternalInput")
    bqkv_d = nc.dram_tensor("bqkv", [D, 3], F32, kind="ExternalInput")
    bvrow_d = nc.dram_tensor("bvrow", [1, D], F32, kind="ExternalInput")
    envs_d = nc.dram_tensor("envs", [128, W], F32, kind="ExternalInput")
    bs_d = nc.dram_tensor("bs", [128, W], F32, kind="ExternalInput")
    ao_d = nc.dram_tensor("ao", [N, F], F32, kind="ExternalOutput")

    dbg = {}
    if _DBG:
        for nm, shp in [
            ("dbg_expsf", [128, N]),
            ("dbg_dt", [128, NT * NSEG]),
            ("dbg_ct", [128, NT * NSEG]),
            ("dbg_dd", [NSEG, N]),
            ("dbg_aggt", [128, N]),
            ("dbg_ao", [128, F]),
            ("dbg_fq", [128, N]),
            ("dbg_fk", [128, N]),
            ("dbg_vhn", [128, F]),
        ]:
            dbg[nm] = nc.dram_tensor(nm, shp, F32, kind="ExternalOutput")

    with tile.TileContext(nc) as tc:
        with (
            tc.tile_pool(name="const", bufs=1) as cpool,
            tc.tile_pool(name="raw", bufs=1) as rawp,
            tc.tile_pool(name="tposed", bufs=3) as tpp,
            tc.tile_pool(name="feat", bufs=1) as featp,
            tc.tile_pool(name="work", bufs=1) as workp,
            tc.tile_pool(name="aop", bufs=4) as aop,
            tc.tile_pool(name="ps", bufs=8, space="PSUM") as psp,
        ):
            def ps_tile(shape):
                return psp.tile(shape, F32, tag="ps", name="ps")

            # ---------------------------------------------------- constants
            ident = cpool.tile([128, 128], F32, tag="ident", name="ident")
            make_identity(nc, ident[:])
            wq_sb = cpool.tile([128, S * D], F32, tag="wq", name="wq")
            wk_sb = cpool.tile([128, S * D], F32, tag="wk", name="wk")
            wv_sb = cpool.tile([128, S * D], F32, tag="wv", name="wv")
            # dram (s,i),o -> sbuf [i, (s,o)]
            for w_sb, w_d in ((wq_sb, wq_d), (wk_sb, wk_d), (wv_sb, wv_d)):
                nc.gpsimd.dma_start(
                    w_sb[:].rearrange("i (s o) -> i s o", s=S).bitcast(F32R),
                    w_d[:].rearrange("(s i) o -> i s o", s=S),
                )
            bqkv_sb = cpool.tile([D, 3], F32, tag="bqkv", name="bqkv")
            nc.gpsimd.dma_start(bqkv_sb[:], bqkv_d[:])
            bvrow_sb = cpool.tile([128, D], F32, tag="bvrow", name="bvrow")
            nc.gpsimd.dma_start(bvrow_sb[:], bvrow_d[0:1, :].to_broadcast([128, D]))
            envs_sb = cpool.tile([128, W], F32, tag="envs", name="envs")
            nc.gpsimd.dma_start(envs_sb[:], envs_d[:])
            bs_sb = cpool.tile([128, W], F32, tag="bs", name="bs")
            nc.gpsimd.dma_start(bs_sb[:], bs_d[:])
            import contextlib as _ctl
            _loop = tc.For_i(0, loop_R, 1) if loop_R else _ctl.nullcontext()
            with _loop:
                # ------------------------------------------- D / C tables (slots)
                ebs = workp.tile([128, W], F32, tag="ebs", name="ebs")
                nc.scalar.activation(ebs[:], bs_sb[:], AF.Exp)
                wD = workp.tile([128, W], F32, tag="wD", name="wD")
                nc.vector.tensor_tensor(out=wD[:], in0=envs_sb[:], in1=ebs[:], op=ALU.mult)
                wC = workp.tile([128, W], F32, tag="wC", name="wC")
                nc.vector.tensor_tensor(out=wC[:], in0=wD[:], in1=envs_sb[:], op=ALU.mult)
                d_t = featp.tile([128, NT * NSEG], F32, tag="d_t", name="d_t")  # [m_p, (mt, g)]
                c_t = featp.tile([128, NT * NSEG], F32, tag="c_t", name="c_t")
                with nc.allow_low_precision(reason="f32r is 32-bit storage"):
                    nc.vector.reduce_sum(
                        out=d_t[:].rearrange("p (t g) -> p t g", t=NT).bitcast(F32R),
                        in_=wD[:].rearrange("p (t g j) -> p t g j", t=NT, g=NSEG),
                        axis=mybir.AxisListType.X,
                    )
                nc.vector.reduce_sum(
                    out=c_t[:].rearrange("p (t g) -> p t g", t=NT),
                    in_=wC[:].rearrange("p (t g j) -> p t g j", t=NT, g=NSEG),
                    axis=mybir.AxisListType.X,
                )
                # C transposed to [g, m]
                c_sb = featp.tile([NSEG, N], F32, tag="c_sb", name="c_sb")
                for mt in range(NT):
                    pc = ps_tile([NSEG, 128])
                    nc.tensor.transpose(
                        pc[:], c_t[:, mt * NSEG:(mt + 1) * NSEG], ident[:]
                    )
                    nc.vector.tensor_copy(out=c_sb[:, mt * 128:(mt + 1) * 128].bitcast(F32R), in_=pc[:])

                # ------------------------- load + transpose + project q, k, v
                # fq/fk chunk tiles: rows (s_local*32+o), chunks s=0..3 / 4..7 / 8
                fq = [featp.tile([128, N], F32, tag="fq0", name="fq0"),
                      featp.tile([128, N], F32, tag="fq1", name="fq1"),
                      featp.tile([D, N], F32, tag="fq2", name="fq2")]
                fk = [featp.tile([128, N], F32, tag="fk0", name="fk0"),
                      featp.tile([128, N], F32, tag="fk1", name="fk1"),
                      featp.tile([D, N], F32, tag="fk2", name="fk2")]
                vhn = [featp.tile([128, F], F32, tag=f"vhn{mt}", name=f"vhn{mt}") for mt in range(NT)]

                copy_engines = [nc.scalar, nc.vector]
                cp_i = 0

                def copy_alt(dst_ap, src_ap):
                    nonlocal cp_i
                    eng = copy_engines[cp_i % 2]
                    cp_i += 1
                    if eng is nc.scalar:
                        eng.copy(dst_ap.bitcast(F32R), src_ap)
                    else:
                        eng.tensor_copy(out=dst_ap.bitcast(F32R), in_=src_ap)

                for t_idx, (t_d, w_sb, f_dst) in enumerate(
                    ((q_d, wq_sb, fq), (k_d, wk_sb, fk), (v_d, wv_sb, None))
                ):
                    t_nm = "qkv"[t_idx]
                    raw_tiles = {}
                    for nt in range(NT):
                        r = rawp.tile([128, S * CIN], F32, tag=f"raw{t_nm}{nt}",
                                      name=f"raw{t_nm}{nt}")
                        eng = nc.sync if (nt % 2 == 0) else nc.scalar
                        eng.dma_start(r[:], t_d[nt * 128:(nt + 1) * 128, :])
                        raw_tiles[(t_nm, nt)] = r
                    # transpose all 9 components into [i, m] tiles; 4 per-nt
                    # transposes land in one PSUM bank -> one wide copy each
                    t_T = [tpp.tile([128, N], F32, tag=f"T{s}", name=f"T{s}") for s in range(S)]
                    for s in range(S):
                        ptile = ps_tile([128, N])
                        for nt in range(NT):
                            nc.tensor.transpose(
                                ptile[:, nt * 128:(nt + 1) * 128],
                                raw_tiles[(t_nm, nt)][:, s * CIN:(s + 1) * CIN],
                                ident[:],
                            )
                        copy_alt(t_T[s][:], ptile[:])
                    if f_dst is not None:
                        # f-major projection: [o, m] per s
                        for s in range(S):
                            pp = ps_tile([D, N])
                            nc.tensor.matmul(
                                pp[:],
                                lhsT=w_sb[:, s * D:(s + 1) * D].bitcast(F32R),
                                rhs=t_T[s][:].bitcast(F32R),
                                start=True, stop=True,
                            )
                            chunk, row = divmod(s, 4)
                            dst = f_dst[chunk][row * D:(row + 1) * D, :]
                            if s == 0:
                                nc.vector.tensor_scalar_add(
                                    dst.bitcast(F32R), pp[:],
                                    bqkv_sb[:, t_idx:t_idx + 1]
                                )
                            else:
                                copy_alt(dst, pp[:])
                    else:
                        # vhn [m, (s, o)]: 9 col-slices share one PSUM bank
                        for mt in range(NT):
                            pv = ps_tile([128, F])
                            for s in range(S):
                                nc.tensor.matmul(
                                    pv[:, s * D:(s + 1) * D],
                                    lhsT=t_T[s][:, mt * 128:(mt + 1) * 128],
                                    rhs=w_sb[:, s * D:(s + 1) * D],
                                    start=True, stop=True,
                                )
                            copy_alt(vhn[mt][:], pv[:])
                            nc.vector.tensor_tensor(
                                out=vhn[mt][:, 0:D].bitcast(F32R),
                                in0=vhn[mt][:, 0:D], in1=bvrow_sb[:], op=ALU.add,
                            )

                if _DBG:
                    nc.sync.dma_start(dbg["dbg_fq"][:], fq[0][:])
                    nc.sync.dma_start(dbg["dbg_fk"][:], fk[0][:])
                    nc.sync.dma_start(dbg["dbg_vhn"][:], vhn[0][:])

                # ---------------------------------------- scores + exp, per m-tile
                exp_sf = [featp.tile([128, N], F32, tag=f"esf{mt}", name=f"esf{mt}") for mt in range(NT)]
                for mt in range(NT):
                    psf = ps_tile([128, N])
                    for c, rows in ((0, 128), (1, 128), (2, D)):
                        nc.tensor.matmul(
                            psf[:],
                            lhsT=fk[c][:rows, mt * 128:(mt + 1) * 128].bitcast(F32R),
                            rhs=fq[c][:rows, :].bitcast(F32R),
                            start=(c == 0), stop=(c == 2),
                        )
                    nc.scalar.activation(exp_sf[mt][:].bitcast(F32R), psf[:], AF.Exp, scale=SCALE)

                # --------------------------------------------- denominators -> dd
                pden = ps_tile([NSEG, N])
                for mt in range(NT):
                    nc.tensor.matmul(
                        pden[:], lhsT=d_t[:, mt * NSEG:(mt + 1) * NSEG].bitcast(F32R),
                        rhs=exp_sf[mt][:].bitcast(F32R),
                        start=(mt == 0), stop=(mt == NT - 1),
                    )
                dde = featp.tile([NSEG, N], F32, tag="dde", name="dde")
                nc.vector.tensor_scalar_add(dde[:], pden[:], 1e-16)
                dd = featp.tile([NSEG, N], F32, tag="dd", name="dd")
                with nc.allow_low_precision(reason="f32r is 32-bit storage"):
                    nc.vector.reciprocal(dd[:].bitcast(F32R), dde[:])

                # --------------------------------------- Aagg^T = exp_sf * (C^T dd)
                aggt = [featp.tile([128, N], F32, tag=f"aggt{mt}", name=f"aggt{mt}") for mt in range(NT)]
                for mt in range(NT):
                    pT = ps_tile([128, N])
                    nc.tensor.matmul(
                        pT[:], lhsT=c_sb[:, mt * 128:(mt + 1) * 128].bitcast(F32R),
                        rhs=dd[:].bitcast(F32R),
                        start=True, stop=True,
                    )
                    nc.vector.tensor_tensor(
                        out=aggt[mt][:].bitcast(F32R), in0=exp_sf[mt][:], in1=pT[:],
                        op=ALU.mult
                    )

                # --------------------------------- attention output, per n-tile
                for nt in range(NT):
                    po = ps_tile([128, F])
                    for mt in range(NT):
                        nc.tensor.matmul(
                            po[:],
                            lhsT=aggt[mt][:, nt * 128:(nt + 1) * 128].bitcast(F32R),
                            rhs=vhn[mt][:].bitcast(F32R),
                            start=(mt == 0), stop=(mt == NT - 1),
                        )
                    ao = aop.tile([128, F], F32, tag="ao", name="ao")
                    nc.scalar.copy(ao[:], po[:])
                    if _DBG and nt == 0:
                        nc.sync.dma_start(dbg["dbg_ao"][:], ao[:])
                    nc.sync.dma_start(ao_d[nt * 128:(nt + 1) * 128, :], ao[:])

                if _DBG:
                    nc.sync.dma_start(dbg["dbg_expsf"][:], exp_sf[0][:])
                    nc.sync.dma_start(dbg["dbg_dt"][:], d_t[:])
                    nc.sync.dma_start(dbg["dbg_ct"][:], c_t[:])
                    nc.sync.dma_start(dbg["dbg_dd"][:], dd[:])
                    nc.sync.dma_start(dbg["dbg_aggt"][:], aggt[0][:])

    _split_multiwaits(nc)
    return nc


def build_phase2(loop_R: int | None = None) -> bass.Bass:
    """Equivariant layernorm + output projection on a 64-atom slice.
    Input lnin [64, (s, ci)] is the host-reordered concat of the per-head
    phase-1 outputs; same NEFF on all cores."""
    nc = bass.Bass("TRN2", target_bir_lowering=False, debug=False, num_devices=H)
    lnin_d = nc.dram_tensor("lnin", [NR, S * CH], F32, kind="ExternalInput")
    grow_d = nc.dram_tensor("grow", [1, S * CH], F32, kind="ExternalInput")
    beta_d = nc.dram_tensor("beta0", [1, CH], F32, kind="ExternalInput")
    woe_d = nc.dram_tensor("woe", [2, 128, S * CIN], F32, kind="ExternalInput")
    bo_d = nc.dram_tensor("bo", [1, CIN], F32, kind="ExternalInput")
    y_d = nc.dram_tensor("y", [NR, S * CIN], F32, kind="ExternalOutput")

    with tile.TileContext(nc) as tc:
        with (
            tc.tile_pool(name="const", bufs=1) as cpool,
            tc.tile_pool(name="work", bufs=1) as workp,
            tc.tile_pool(name="tp", bufs=4) as tpp,
            tc.tile_pool(name="ps", bufs=8, space="PSUM") as psp,
        ):
            def ps_tile(shape):
                return psp.tile(shape, F32, tag="ps", name="ps")

            ident = cpool.tile([128, 128], F32, tag="ident", name="ident")
            make_identity(nc, ident[:])
            eps_sb = cpool.tile([128, 1], F32, tag="epsc", name="epsc")
            nc.gpsimd.memset(eps_sb[:], EPS)
            grow_sb = cpool.tile([NR, S * CH], F32, tag="grow", name="grow")
            nc.gpsimd.dma_start(grow_sb[:], grow_d[0:1, :].to_broadcast([NR, S * CH]))
            beta_sb = cpool.tile([NR, CH], F32, tag="beta", name="beta")
            nc.gpsimd.dma_start(beta_sb[:], beta_d[0:1, :].to_broadcast([NR, CH]))
            bo_sb = cpool.tile([NR, CIN], F32, tag="bo", name="bo")
            nc.gpsimd.dma_start(bo_sb[:], bo_d[0:1, :].to_broadcast([NR, CIN]))
            woe_sb = [
                cpool.tile([128, S * CIN], F32, tag=f"woe{c}", name=f"woe{c}")
                for c in range(2)
            ]
            for c in range(2):
                nc.gpsimd.dma_start(woe_sb[c][:], woe_d[c, :, :])
            import contextlib as _ctl
            _loop = tc.For_i(0, loop_R, 1) if loop_R else _ctl.nullcontext()
            with _loop:
                lnin = workp.tile([NR, S * CH], F32, tag="lnin", name="lnin")
                nc.sync.dma_start(lnin[:], lnin_d[:])
                lnout = workp.tile([NR, S * CH], F32, tag="lnout", name="lnout")

                x0 = lnin[:, 0:CH]
                sc0 = workp.tile([NR, CH], F32, tag="sc0", name="sc0")
                mu = workp.tile([NR, 1], F32, tag="mu", name="mu")
                nc.scalar.activation(sc0[:], x0, AF.Copy, scale=1.0 / CH,
                                     accum_out=mu[:])
                nmu = workp.tile([NR, 1], F32, tag="nmu", name="nmu")
                nc.scalar.mul(nmu[:], mu[:], -1.0)
                xc = workp.tile([NR, CH], F32, tag="xc", name="xc")
                nc.scalar.activation(xc[:], x0, AF.Identity, bias=nmu[:, 0:1])
                vs = workp.tile([NR, 1], F32, tag="vs", name="vs")
                sq0 = workp.tile([NR, CH], F32, tag="sq0", name="sq0")
                nc.scalar.activation(sq0[:], xc[:], AF.Square, accum_out=vs[:])
                sd = workp.tile([NR, 1], F32, tag="sd", name="sd")
                nc.scalar.activation(sd[:], vs[:], AF.Sqrt, scale=1.0 / CH,
                                     bias=eps_sb[0:NR, 0:1])
                rstd = workp.tile([NR, 1], F32, tag="rstd", name="rstd")
                nc.vector.reciprocal(rstd[:], sd[:])
                nc.scalar.activation(lnout[:, 0:CH], xc[:], AF.Copy,
                                     scale=rstd[:, 0:1])
                for l in (1, 2):
                    lo, hi = (l * l) * CH, ((l + 1) * (l + 1)) * CH
                    width = hi - lo
                    sql = workp.tile([NR, width], F32, tag=f"sq{l}", name=f"sq{l}")
                    ms = workp.tile([NR, 1], F32, tag=f"ms{l}", name=f"ms{l}")
                    nc.scalar.activation(sql[:], lnin[:, lo:hi], AF.Square,
                                         accum_out=ms[:])
                    sdl = workp.tile([NR, 1], F32, tag=f"sd{l}", name=f"sd{l}")
                    nc.scalar.activation(sdl[:], ms[:], AF.Sqrt, scale=1.0 / width,
                                         bias=eps_sb[0:NR, 0:1])
                    rrl = workp.tile([NR, 1], F32, tag=f"rr{l}", name=f"rr{l}")
                    nc.vector.reciprocal(rrl[:], sdl[:])
                    nc.vector.tensor_scalar_mul(lnout[:, lo:hi], lnin[:, lo:hi],
                                                rrl[:, 0:1])
                # gamma (per component) then beta (l=0 only)
                nc.vector.tensor_tensor(
                    out=lnout[:], in0=lnout[:], in1=grow_sb[:], op=ALU.mult,
                )
                nc.vector.tensor_tensor(
                    out=lnout[:, 0:CH], in0=lnout[:, 0:CH], in1=beta_sb[:],
                    op=ALU.add,
                )

                y_sb = workp.tile([NR, S * CIN], F32, tag="ysb", name="ysb")
                for s in range(S):
                    py = ps_tile([NR, CIN])
                    for c in range(2):
                        pl = ps_tile([128, NR])
                        nc.tensor.transpose(
                            pl[:], lnout[:, s * CH + c * 128: s * CH + (c + 1) * 128],
                            ident[0:NR, 0:NR],
                        )
                        lnT = tpp.tile([128, NR], F32, tag="lnT", name="lnT")
                        nc.vector.tensor_copy(out=lnT[:], in_=pl[:])
                        nc.tensor.matmul(
                            py[:], lhsT=lnT[:],
                            rhs=woe_sb[c][:, s * CIN:(s + 1) * CIN],
                            start=(c == 0), stop=(c == 1),
                        )
                    nc.vector.tensor_tensor(
                        out=y_sb[:, s * CIN:(s + 1) * CIN], in0=py[:],
                        in1=bo_sb[:], op=ALU.add,
                    )
                nc.sync.dma_start(y_d[:], y_sb[:])

    _split_multiwaits(nc)
    return nc


# ------------------------------------------------------------------ host side
def _prep_inputs(inputs: dict[str, np.ndarray]):
    """Split the full inputs into per-core in_maps for the two phases
    (index bookkeeping and value re-layout only; all arithmetic on device)."""
    q = np.ascontiguousarray(np.asarray(inputs["q"], np.float32).reshape(N, S * CIN))
    k = np.ascontiguousarray(np.asarray(inputs["k"], np.float32).reshape(N, S * CIN))
    v = np.ascontiguousarray(np.asarray(inputs["v"], np.float32).reshape(N, S * CIN))
    env = np.asarray(inputs["envelope"], np.float32)
    attn_bias = np.asarray(inputs["attn_bias"], np.float32)
    a_idx = np.asarray(inputs["atom_index"]).astype(np.int64)
    b_idx = np.asarray(inputs["batch_index"]).astype(np.int64)
    e_map = np.asarray(inputs["edge_map_tab"]).astype(np.int64)
    Wq = np.asarray(inputs["Wq"], np.float32)
    Wk = np.asarray(inputs["Wk"], np.float32)
    Wv = np.asarray(inputs["Wv"], np.float32)
    bq = np.asarray(inputs["bq"], np.float32)
    bk = np.asarray(inputs["bk"], np.float32)
    bv = np.asarray(inputs["bv"], np.float32)
    gamma = np.asarray(inputs["gamma"], np.float32)
    beta = np.asarray(inputs["beta"], np.float32)
    Wo = np.asarray(inputs["Wo"], np.float32)
    bo = np.asarray(inputs["bo"], np.float32)

    # ---- slot layout for the (atom, segment) cells
    cell = a_idx * NSEG + b_idx                      # [E]
    order = np.argsort(cell, kind="stable")
    cell_s = cell[order]
    counts = np.bincount(cell_s, minlength=N * NSEG)
    L2 = int(counts.max())
    starts = np.zeros(N * NSEG, np.int64)
    starts[1:] = np.cumsum(counts)[:-1]
    rank = np.arange(E) - starts[cell_s]             # rank within cell
    m_s = cell_s // NSEG
    g_s = cell_s % NSEG
    p_s = m_s % 128
    t_s = m_s // 128
    col = (t_s * NSEG + g_s) * L2 + rank             # free-dim position
    Wd = NT * NSEG * L2
    env_e = env[e_map]                               # value gather (re-layout)
    envS = np.zeros((128, Wd), np.float32)
    envS[p_s, col] = env_e[order]
    bS_all = []
    for h in range(H):
        bs = np.zeros((128, Wd), np.float32)
        bs[p_s, col] = attn_bias[h, e_map][order]
        bS_all.append(bs)

    # ---- per-head weight slices, expanded per spherical component
    WqE = Wq[L_OF_M]                                 # [9, CIN, CH]
    WkE = Wk[L_OF_M]
    WvE = Wv[L_OF_M]
    gE = gamma[L_OF_M]                               # [9, CH]
    WoE = Wo[L_OF_M]                                 # [9, CH, CIN]

    grow = np.ascontiguousarray(gE.reshape(1, S * CH))
    woe = np.zeros((2, 128, S * CIN), np.float32)
    for c in range(2):
        woe[c] = (
            WoE[:, c * 128:(c + 1) * 128, :].transpose(1, 0, 2).reshape(128, S * CIN)
        )
    beta0 = np.ascontiguousarray(beta.reshape(1, CH))
    boR = np.ascontiguousarray(bo.reshape(1, CIN))

    in_maps1 = []
    for h in range(H):
        sl = slice(h * D, (h + 1) * D)
        in_maps1.append({
            "q": q, "k": k, "v": v,
            "wq": np.ascontiguousarray(WqE[:, :, sl].reshape(S * CIN, D)),
            "wk": np.ascontiguousarray(WkE[:, :, sl].reshape(S * CIN, D)),
            "wv": np.ascontiguousarray(WvE[:, :, sl].reshape(S * CIN, D)),
            "bqkv": np.ascontiguousarray(
                np.stack([bq[sl], bk[sl], bv[sl]], axis=1)
            ),
            "bvrow": np.ascontiguousarray(bv[sl].reshape(1, D)),
            "envs": envS,
            "bs": bS_all[h],
        })
    p2_const = {"grow": grow, "beta0": beta0, "woe": woe, "bo": boR}
    return in_maps1, L2, p2_const


def _reorder_ao(ao_all: list[np.ndarray]) -> list[np.ndarray]:
    """[h][N, (s,d)] -> per-core [64, (s, h*D+d)] slices (pure data movement)."""
    full = np.stack([a.reshape(N, S, D) for a in ao_all], axis=2)  # [N, S, H, D]
    full = full.reshape(N, S * CH)
    return [np.ascontiguousarray(full[c * NR:(c + 1) * NR]) for c in range(H)]


_BUILD_CACHE: dict = {}


def kernel(**inputs) -> np.ndarray:
    in_maps1, L2, p2_const = _prep_inputs(inputs)
    nc1 = _BUILD_CACHE.get(("p1", L2))
    if nc1 is None:
        nc1 = build_bass(L2)
        _BUILD_CACHE[("p1", L2)] = nc1
    res1 = run_bass_kernel_spmd(nc1, in_maps1, core_ids=list(range(H)))
    lnin_slices = _reorder_ao([r["ao"] for r in res1.results])

    nc2 = _BUILD_CACHE.get("p2")
    if nc2 is None:
        nc2 = build_phase2()
        _BUILD_CACHE["p2"] = nc2
    in_maps2 = [{"lnin": lnin_slices[c], **p2_const} for c in range(H)]
    res2 = run_bass_kernel_spmd(nc2, in_maps2, core_ids=list(range(H)))
    y = np.concatenate([r["y"] for r in res2.results], axis=0)
    return np.ascontiguousarray(y.reshape(N, S, CIN).astype(np.float32))



# revision 41
# speedup vs baseline: 1.9995x; 1.9995x over previous
"""Equivariant attention (gnn_message_passing) on 8 Trainium2 NeuronCores.

Strategy (head-sharded tensor parallel, core c owns head c):

The reference materializes [H, N, E] scores/attn over E=8192 edges. Here the
edge dimension is collapsed onto the N=512 atoms at projection level:

  scores[h, n, e]   = sf[h, n, a_e] + bias[h, edge_map[e]]     (a_e = atom_index)
  attn-softmax per (batch-segment, n) then  out = attn @ vh_edges

factors exactly into dense [N, N] algebra with two tiny per-(segment, atom)
tables (NSEG=16 x N=512):

  D[g, m] = sum_{e in seg g, a_e = m} env_e   * exp(b_e)
  C[g, m] = sum_{e in seg g, a_e = m} env_e^2 * exp(b_e)
  den[g, n]  = sum_m exp(sf[m, n]) * D[g, m]           (one matmul)
  Aagg[m, n] = exp(sf[m, n]) * sum_g C[g, m] / den[g, n]
  out[n, f]  = Aagg^T @ vh[m, f]                        (one matmul)

The running-max subtraction in the reference softmax cancels exactly (up to a
+1e-16 epsilon whose relative effect is ~1e-16) and |scale*sf + b| < 20, so
unnormalized exp is safe in f32.

D/C are built on-device from "slot tensors": host packs per-(atom, segment)
edge lists into a fixed-width [128, 4*16*L2] layout (env & bias values; pads
have env=0 so they vanish), and a single free-axis reduce per table produces
it. Only integer index bookkeeping and value re-layout happen on host.

q/k/v arrive HOST-PRE-TRANSPOSED as qT/kT/vT [CIN, S*N] (channel-major), so
the kernel needs no on-device input transposes: projections read qT slices
directly.  DMAs are issued in first-use order so the PE starts ~1.5us in.

Phase 2 (per-core 64-atom slice): LN + output projection, with the output
projection done in yT [ci, (s, n)] orientation (gamma/beta folded into the
post-transpose PSUM copy); host un-transposes for free.
"""

import os
import numpy as np

import concourse.bass as bass
import concourse.tile as tile
from concourse import mybir
from concourse.bass_utils import run_bass_kernel_spmd
from concourse.masks import make_identity

# ---------------------------------------------------------------- constants
H, LMAX, NSEG = 8, 2, 16
S = (LMAX + 1) ** 2          # 9 spherical components
N, E, CIN, CH = 512, 8192, 128, 256
D = CH // H                  # 32 per-head channels
F = S * D                    # 288 per-head feature width
NT = N // 128                # 4 atom tiles
NR = N // H                  # 64 atoms per core in the LN/out stage
EPS = 1e-7
SCALE = float(np.sqrt(D / 3.0) / D)
L_OF_M = np.floor(np.sqrt(np.arange(S))).astype(np.int64)
F32 = mybir.dt.float32
F32R = mybir.dt.float32r
BF16 = mybir.dt.bfloat16
AF = mybir.ActivationFunctionType
ALU = mybir.AluOpType

import ml_dtypes
NP_BF16 = ml_dtypes.bfloat16

_DBG = bool(int(os.environ.get("KBDBG", "0")))


def _split_multiwaits(nc: bass.Bass, limit: int = 1):
    """This walrus build rejects instructions carrying more than one semaphore
    wait (and Drains carrying any). Hoist excess waits onto NOPs inserted just
    before the instruction on the same engine - semantically identical."""
    for f in nc.m.functions:
        for blk in f.blocks:
            changed = False
            out = []
            for inst in blk.instructions:
                si = inst.sync_info
                waits = list(si.on_wait) if si is not None else []
                keep = 0 if inst.opcode == "Drain" else limit
                if len(waits) > keep:
                    hoist = waits[: len(waits) - keep]
                    rest = waits[len(waits) - keep:]
                    for w in hoist:
                        nop = mybir.InstNoOp(
                            name=f"{inst.name}-w{len(out)}", ins=[], outs=[]
                        )
                        nop.engine = inst.engine
                        nop.sync_info = mybir.SyncInfo(on_wait=[w], on_update=[])
                        out.append(nop)
                    inst.sync_info = mybir.SyncInfo(
                        on_wait=rest, on_update=list(si.on_update)
                    )
                    changed = True
                out.append(inst)
            if changed:
                blk.instructions = out


def build_bass(L2: int, loop_R: int | None = None) -> bass.Bass:
    """One SPMD program; per-core data (weight slices, bias slots) comes in as
    inputs. L2 = slot width per (atom, segment) cell."""
    W = NT * NSEG * L2  # slot tensor free width per partition

    nc = bass.Bass("TRN2", target_bir_lowering=False, debug=False, num_devices=H)

    # ------------------------------------------------------------- tensors
    # host-pre-transposed bf16 inputs: qT/kT [i, (s, m)]; vT mt-major
    # [i, (t, s, j)] so each quarter is a contiguous DMA
    qT_d = nc.dram_tensor("qT", [CIN, S * N], BF16, kind="ExternalInput")
    kT_d = nc.dram_tensor("kT", [CIN, S * N], BF16, kind="ExternalInput")
    vT_d = nc.dram_tensor("vT", [CIN, S * N], BF16, kind="ExternalInput")
    wq_d = nc.dram_tensor("wq", [CIN, S * D], BF16, kind="ExternalInput")  # [i,(s,o)]
    wk_d = nc.dram_tensor("wk", [CIN, S * D], BF16, kind="ExternalInput")
    wv_d = nc.dram_tensor("wv", [CIN, S * D], BF16, kind="ExternalInput")
    bqkv_d = nc.dram_tensor("bqkv", [D, 3], F32, kind="ExternalInput")
    bvrow_d = nc.dram_tensor("bvrow", [1, D], F32, kind="ExternalInput")
    envs_d = nc.dram_tensor("envs", [128, W], F32, kind="ExternalInput")
    bs_d = nc.dram_tensor("bs", [128, W], F32, kind="ExternalInput")
    ao_d = nc.dram_tensor("ao", [N, F], BF16, kind="ExternalOutput")

    with tile.TileContext(nc) as tc:
        with (
            tc.tile_pool(name="const", bufs=1) as cpool,
            tc.tile_pool(name="feat", bufs=1) as featp,
            tc.tile_pool(name="work", bufs=1) as workp,
            tc.tile_pool(name="aop", bufs=1) as aop,
            tc.tile_pool(name="ps", bufs=4, space="PSUM") as psp,
            tc.tile_pool(name="pop", bufs=1, space="PSUM") as pop,
        ):
            def ps_tile(shape):
                return psp.tile(shape, F32, tag="ps", name="ps")

            # -------- DMA issue, first-use order, spread over engines ------
            # sync queue:   wq, qT thirds, vT quarters (mt-major layout)
            # scalar queue: wk, wv, kT thirds, envs, bs
            # gpsimd queue: tiny constants
            TH = 3 * N
            wq_sb = cpool.tile([CIN, S * D], BF16, tag="wq", name="wq")
            wk_sb = cpool.tile([CIN, S * D], BF16, tag="wk", name="wk")
            wv_sb = cpool.tile([CIN, S * D], BF16, tag="wv", name="wv")
            qTc = [cpool.tile([CIN, TH], BF16, tag=f"qT{t}", name=f"qT{t}")
                   for t in range(3)]
            kTc = [cpool.tile([CIN, TH], BF16, tag=f"kT{t}", name=f"kT{t}")
                   for t in range(3)]
            # vT quarter mt holds [i, (s, m_local)] for atom tile mt
            vTq = [cpool.tile([CIN, S * 128], BF16, tag=f"vT{m}", name=f"vT{m}")
                   for m in range(NT)]
            envs_sb = cpool.tile([128, W], F32, tag="envs", name="envs")
            bs_sb = cpool.tile([128, W], F32, tag="bs", name="bs")

            nc.sync.dma_start(wq_sb[:], wq_d[:])
            for t in range(3):
                nc.sync.dma_start(qTc[t][:], qT_d[:, t * TH:(t + 1) * TH])
            for m in range(NT):
                nc.sync.dma_start(
                    vTq[m][:],
                    vT_d[:].rearrange("i (t x) -> i t x", t=NT)[:, m, :],
                )
            nc.scalar.dma_start(wk_sb[:], wk_d[:])
            for t in range(3):
                nc.scalar.dma_start(kTc[t][:], kT_d[:, t * TH:(t + 1) * TH])
            nc.scalar.dma_start(envs_sb[:], envs_d[:])
            nc.scalar.dma_start(bs_sb[:], bs_d[:])
            nc.scalar.dma_start(wv_sb[:], wv_d[:])
            bqkv_sb = cpool.tile([D, 3], F32, tag="bqkv", name="bqkv")
            nc.gpsimd.dma_start(bqkv_sb[:], bqkv_d[:])
            bvrow_sb = cpool.tile([128, D], F32, tag="bvrow", name="bvrow")
            nc.gpsimd.dma_start(bvrow_sb[:], bvrow_d[0:1, :].to_broadcast([128, D]))
            ident = cpool.tile([128, 128], F32, tag="ident", name="ident")
            make_identity(nc, ident[:])

            import contextlib as _ctl
            _loop = tc.For_i(0, loop_R, 1) if loop_R else _ctl.nullcontext()
            with _loop:
                copy_engines = [nc.scalar, nc.vector]
                cp_i = 0

                def copy_alt(dst_ap, src_ap):
                    nonlocal cp_i
                    eng = copy_engines[cp_i % 2]
                    cp_i += 1
                    if eng is nc.scalar:
                        eng.copy(dst_ap, src_ap)
                    else:
                        eng.tensor_copy(out=dst_ap, in_=src_ap)

                # ------- fq / fk chunk-streamed projection + psf accumulation
                # chunk layout: rows (s_local*32+o), chunks s=0..2 / 3..5 / 6..8
                # (96 rows per chunk so matmul outs land at base 0/32/64);
                # psf[mt] accumulates across chunks in 4 held banks (tags
                # shared with the po accumulators, which start strictly later)
                fq = [featp.tile([96, N], BF16, tag=f"fq{c}", name=f"fq{c}")
                      for c in range(3)]
                fk = [featp.tile([96, N], BF16, tag=f"fk{c}", name=f"fk{c}")
                      for c in range(3)]
                psf = [pop.tile([128, N], F32, tag=f"acc{mt}", name=f"psf{mt}")
                       for mt in range(NT)]
                for chunk in range(3):
                    for t_c, w_sb, f_dst, t_idx in ((qTc, wq_sb, fq, 0),
                                                    (kTc, wk_sb, fk, 1)):
                        pp = ps_tile([96, N])
                        for j in range(3):
                            s = chunk * 3 + j
                            nc.tensor.matmul(
                                pp[j * D:(j + 1) * D, :],
                                lhsT=w_sb[:, s * D:(s + 1) * D],
                                rhs=t_c[chunk][:, j * N:(j + 1) * N],
                                start=True, stop=True,
                            )
                        if chunk == 0:
                            # bias on s=0 rows (l=0 invariant component);
                            # PSUM reads >32 partitions must be 64-aligned
                            nc.vector.tensor_scalar_add(
                                f_dst[0][0:D, :], pp[0:D, :],
                                bqkv_sb[:, t_idx:t_idx + 1])
                            copy_alt(f_dst[0][D:64, :], pp[D:64, :])
                            copy_alt(f_dst[0][64:96, :], pp[64:96, :])
                        else:
                            copy_alt(f_dst[chunk][:], pp[:])
                    for mt in range(NT):
                        nc.tensor.matmul(
                            psf[mt][:],
                            lhsT=fk[chunk][:, mt * 128:(mt + 1) * 128],
                            rhs=fq[chunk][:],
                            start=(chunk == 0), stop=(chunk == 2),
                            skip_group_check=True,
                        )

                # ------------------------------ D / C tables (early, off-path)
                ebs = workp.tile([128, W], F32, tag="ebs", name="ebs")
                nc.scalar.activation(ebs[:], bs_sb[:], AF.Exp)
                wD = workp.tile([128, W], F32, tag="wD", name="wD")
                nc.vector.tensor_tensor(out=wD[:], in0=envs_sb[:], in1=ebs[:], op=ALU.mult)
                wC = workp.tile([128, W], F32, tag="wC", name="wC")
                nc.vector.tensor_tensor(out=wC[:], in0=wD[:], in1=envs_sb[:], op=ALU.mult)
                d_t = featp.tile([128, NT * NSEG], F32, tag="d_t", name="d_t")  # [m_p, (mt, g)]
                c_t = featp.tile([128, NT * NSEG], F32, tag="c_t", name="c_t")
                with nc.allow_low_precision(reason="f32r is 32-bit storage"):
                    nc.vector.reduce_sum(
                        out=d_t[:].rearrange("p (t g) -> p t g", t=NT).bitcast(F32R),
                        in_=wD[:].rearrange("p (t g j) -> p t g j", t=NT, g=NSEG),
                        axis=mybir.AxisListType.X,
                    )
                nc.vector.reduce_sum(
                    out=c_t[:].rearrange("p (t g) -> p t g", t=NT),
                    in_=wC[:].rearrange("p (t g j) -> p t g j", t=NT, g=NSEG),
                    axis=mybir.AxisListType.X,
                )
                # C transposed to [g, m]
                c_sb = featp.tile([NSEG, N], F32, tag="c_sb", name="c_sb")
                for mt in range(NT):
                    pc = ps_tile([NSEG, 128])
                    nc.tensor.transpose(
                        pc[:], c_t[:, mt * NSEG:(mt + 1) * NSEG], ident[:]
                    )
                    nc.vector.tensor_copy(out=c_sb[:, mt * 128:(mt + 1) * 128].bitcast(F32R), in_=pc[:])

                # -------------------------------- vhn [m, (s,o)] per m-tile
                vhn = [featp.tile([128, F], BF16, tag=f"vhn{mt}", name=f"vhn{mt}") for mt in range(NT)]
                for mt in range(NT):
                    pv = ps_tile([128, F])
                    for s in range(S):
                        nc.tensor.matmul(
                            pv[:, s * D:(s + 1) * D],
                            lhsT=vTq[mt][:, s * 128:(s + 1) * 128],
                            rhs=wv_sb[:, s * D:(s + 1) * D],
                            start=True, stop=True,
                        )
                    nc.vector.tensor_scalar_add(vhn[mt][:, D:F], pv[:, D:F], 0.0)
                    nc.vector.tensor_tensor(
                        out=vhn[mt][:, 0:D],
                        in0=pv[:, 0:D], in1=bvrow_sb[:], op=ALU.add,
                    )

                # ---------------- exp + denominator accumulation, staggered
                exp_sf = [featp.tile([128, N], F32, tag=f"esf{mt}", name=f"esf{mt}") for mt in range(NT)]
                pden = ps_tile([NSEG, N])
                for mt in range(NT):
                    nc.scalar.activation(exp_sf[mt][:].bitcast(F32R), psf[mt][:],
                                         AF.Exp, scale=SCALE)
                    nc.tensor.matmul(
                        pden[:], lhsT=d_t[:, mt * NSEG:(mt + 1) * NSEG].bitcast(F32R),
                        rhs=exp_sf[mt][:].bitcast(F32R),
                        start=(mt == 0), stop=(mt == NT - 1),
                        skip_group_check=True,
                    )
                dde = featp.tile([NSEG, N], F32, tag="dde", name="dde")
                nc.vector.tensor_scalar_add(dde[:], pden[:], 1e-16)
                dd = featp.tile([NSEG, N], F32, tag="dd", name="dd")
                with nc.allow_low_precision(reason="f32r is 32-bit storage"):
                    nc.vector.reciprocal(dd[:].bitcast(F32R), dde[:])

                # ------- per m-tile: aggt = exp_sf * (C^T dd); vhn; att-out
                # po[nt] accumulates mt-major in the acc banks (freed by exp)
                aggt = [featp.tile([128, N], BF16, tag=f"aggt{mt}", name=f"aggt{mt}") for mt in range(NT)]
                po = [pop.tile([128, F], F32, tag=f"acc{nt}", name=f"po{nt}")
                      for nt in range(NT)]
                agg_engines = [nc.vector, nc.vector]
                for mt in range(NT):
                    pT = ps_tile([128, N])
                    nc.tensor.matmul(
                        pT[:], lhsT=c_sb[:, mt * 128:(mt + 1) * 128].bitcast(F32R),
                        rhs=dd[:].bitcast(F32R),
                        start=True, stop=True,
                    )
                    agg_engines[mt % 2].tensor_tensor(
                        out=aggt[mt][:], in0=exp_sf[mt][:],
                        in1=pT[:], op=ALU.mult)
                    for nt in range(NT):
                        nc.tensor.matmul(
                            po[nt][:],
                            lhsT=aggt[mt][:, nt * 128:(nt + 1) * 128],
                            rhs=vhn[mt][:],
                            start=(mt == 0), stop=(mt == NT - 1),
                            skip_group_check=True,
                        )
                ao = aop.tile([128, NT * F], BF16, tag="ao", name="ao")
                for nt in range(NT):
                    copy_alt(ao[:, nt * F:(nt + 1) * F], po[nt][:])
                nc.sync.dma_start(
                    ao_d[:].rearrange("(t p) f -> p t f", t=NT),
                    ao[:].rearrange("p (t f) -> p t f", t=NT))

    _split_multiwaits(nc)
    return nc


def build_phase2(loop_R: int | None = None) -> bass.Bass:
    """Equivariant layernorm + output projection on a 64-atom slice.
    Input lnin [64, (s, ch)]; output yT [ci, (s, n)] (host un-transposes).
    gamma/beta are folded into the post-transpose PSUM->SBUF copies as
    per-partition tensor_scalar ops; same NEFF on all cores."""
    nc = bass.Bass("TRN2", target_bir_lowering=False, debug=False, num_devices=H)
    lnin_d = nc.dram_tensor("lnin", [NR, S * CH], BF16, kind="ExternalInput")
    gcol_d = nc.dram_tensor("gcol", [128, 2 * (LMAX + 1)], F32, kind="ExternalInput")
    bcol_d = nc.dram_tensor("bcol", [128, 2], F32, kind="ExternalInput")
    # compact per-l output weights: [c_half, i, (l, ci)]
    woe_d = nc.dram_tensor("woe", [2, 128, (LMAX + 1) * CIN], BF16, kind="ExternalInput")
    bo_d = nc.dram_tensor("bo", [CIN, 1], F32, kind="ExternalInput")
    y_d = nc.dram_tensor("yT", [CIN, S * NR], F32, kind="ExternalOutput")

    with tile.TileContext(nc) as tc:
        with (
            tc.tile_pool(name="const", bufs=1) as cpool,
            tc.tile_pool(name="work", bufs=1) as workp,
            tc.tile_pool(name="tp", bufs=4) as tpp,
            tc.tile_pool(name="ps", bufs=4, space="PSUM") as psp,
        ):
            def ps_tile(shape):
                return psp.tile(shape, F32, tag="ps", name="ps")

            lnin = workp.tile([NR, S * CH], BF16, tag="lnin", name="lnin")
            # section DMAs so the l=0 chain starts early
            nc.sync.dma_start(lnin[:, 0:CH], lnin_d[:, 0:CH])
            nc.sync.dma_start(lnin[:, CH:4 * CH], lnin_d[:, CH:4 * CH])
            nc.sync.dma_start(lnin[:, 4 * CH:S * CH], lnin_d[:, 4 * CH:S * CH])
            woe_sb = [
                cpool.tile([128, (LMAX + 1) * CIN], BF16, tag=f"woe{c}", name=f"woe{c}")
                for c in range(2)
            ]
            for c in range(2):
                nc.scalar.dma_start(woe_sb[c][:], woe_d[c, :, :])
            gcol_sb = cpool.tile([128, 2 * (LMAX + 1)], F32, tag="gcol", name="gcol")
            nc.gpsimd.dma_start(gcol_sb[:], gcol_d[:])
            bcol_sb = cpool.tile([128, 2], F32, tag="bcol", name="bcol")
            nc.gpsimd.dma_start(bcol_sb[:], bcol_d[:])
            bo_sb = cpool.tile([CIN, 1], F32, tag="bo", name="bo")
            nc.gpsimd.dma_start(bo_sb[:], bo_d[:])
            ident = cpool.tile([128, 128], BF16, tag="ident", name="ident")
            make_identity(nc, ident[:])
            eps_sb = cpool.tile([128, 1], F32, tag="epsc", name="epsc")
            nc.gpsimd.memset(eps_sb[:], EPS)

            import contextlib as _ctl
            _loop = tc.For_i(0, loop_R, 1) if loop_R else _ctl.nullcontext()
            with _loop:
                lnout = workp.tile([NR, S * CH], BF16, tag="lnout", name="lnout")

                # ---- l=0: one-pass LN over CH: var = E[x^2] - mu^2
                x0 = lnin[:, 0:CH]
                sc0 = workp.tile([NR, CH], F32, tag="sc0", name="sc0")
                mu = workp.tile([NR, 1], F32, tag="mu", name="mu")
                nc.scalar.activation(sc0[:], x0, AF.Copy, scale=1.0 / CH,
                                     accum_out=mu[:])
                sq0 = workp.tile([NR, CH], F32, tag="sq0", name="sq0")
                vs = workp.tile([NR, 1], F32, tag="vs", name="vs")
                nc.vector.tensor_tensor(out=sq0[:], in0=x0, in1=x0, op=ALU.mult)
                nc.vector.reduce_sum(out=vs[:], in_=sq0[:],
                                     axis=mybir.AxisListType.X)
                mu2 = workp.tile([NR, 1], F32, tag="mu2", name="mu2")
                nc.vector.tensor_tensor(out=mu2[:], in0=mu[:], in1=mu[:], op=ALU.mult)
                ebias = workp.tile([NR, 1], F32, tag="ebias", name="ebias")
                nc.vector.tensor_scalar(out=ebias[:], in0=mu2[:], scalar1=-1.0,
                                        scalar2=EPS, op0=ALU.mult, op1=ALU.add)
                sd = workp.tile([NR, 1], F32, tag="sd", name="sd")
                nc.scalar.activation(sd[:], vs[:], AF.Sqrt, scale=1.0 / CH,
                                     bias=ebias[:, 0:1])
                rstd = workp.tile([NR, 1], F32, tag="rstd", name="rstd")
                nc.vector.reciprocal(rstd[:], sd[:])
                mr = workp.tile([NR, 1], F32, tag="mr", name="mr")
                nc.vector.tensor_tensor(out=mr[:], in0=mu[:], in1=rstd[:], op=ALU.mult)
                nmr = workp.tile([NR, 1], F32, tag="nmr", name="nmr")
                nc.vector.tensor_scalar_mul(nmr[:], mr[:], -1.0)
                nc.scalar.activation(lnout[:, 0:CH], x0, AF.Identity,
                                     scale=rstd[:, 0:1], bias=nmr[:, 0:1])
                # ---- l=1 / l=2: RMS-of-irrep-norm scaling
                for l in (1, 2):
                    lo, hi = (l * l) * CH, ((l + 1) * (l + 1)) * CH
                    width = hi - lo
                    sql = workp.tile([NR, width], F32, tag=f"sq{l}", name=f"sq{l}")
                    ms = workp.tile([NR, 1], F32, tag=f"ms{l}", name=f"ms{l}")
                    if l == 1:
                        nc.vector.tensor_tensor(out=sql[:], in0=lnin[:, lo:hi],
                                                in1=lnin[:, lo:hi], op=ALU.mult)
                        nc.vector.reduce_sum(out=ms[:], in_=sql[:],
                                             axis=mybir.AxisListType.X)
                    else:
                        nc.scalar.activation(sql[:], lnin[:, lo:hi], AF.Square,
                                             accum_out=ms[:])
                    sdl = workp.tile([NR, 1], F32, tag=f"sd{l}", name=f"sd{l}")
                    nc.scalar.activation(sdl[:], ms[:], AF.Sqrt, scale=1.0 / width,
                                         bias=eps_sb[0:NR, 0:1])
                    rrl = workp.tile([NR, 1], F32, tag=f"rr{l}", name=f"rr{l}")
                    nc.vector.reciprocal(rrl[:], sdl[:])
                    if l == 1:
                        nc.vector.tensor_scalar_mul(lnout[:, lo:hi], lnin[:, lo:hi],
                                                    rrl[:, 0:1])
                    else:
                        nc.scalar.activation(lnout[:, lo:hi], lnin[:, lo:hi],
                                             AF.Copy, scale=rrl[:, 0:1])

                # ---- transpose lnout per (s, c-half); fold gamma (+beta on
                # s=0) into the PSUM->SBUF copy; then yT projection (bf16)
                y_sb = workp.tile([CIN, S * NR], F32, tag="ysb", name="ysb")
                out_dma = [nc.sync, nc.scalar]
                for s in range(S):
                    l = int(L_OF_M[s])
                    py = ps_tile([CIN, NR])
                    for c in range(2):
                        pl = psp.tile([128, NR], BF16, tag="plb", name="plb")
                        nc.tensor.transpose(
                            pl[:], lnout[:, s * CH + c * 128: s * CH + (c + 1) * 128],
                            ident[0:NR, 0:NR],
                        )
                        lnT = tpp.tile([128, NR], BF16, tag="lnT", name="lnT")
                        gc = gcol_sb[:, 2 * l + c:2 * l + c + 1]
                        nc.vector.tensor_scalar(
                            out=lnT[:], in0=pl[:], scalar1=gc,
                            scalar2=bcol_sb[:, c:c + 1] if s == 0 else 0.0,
                            op0=ALU.mult, op1=ALU.add)
                        nc.tensor.matmul(
                            py[:], lhsT=woe_sb[c][:, l * CIN:(l + 1) * CIN],
                            rhs=lnT[:],
                            start=(c == 0), stop=(c == 1),
                        )
                    nc.scalar.activation(y_sb[:, s * NR:(s + 1) * NR], py[:],
                                         AF.Identity, bias=bo_sb[:, 0:1])
                    if s % 3 == 2:
                        lo, hi = (s - 2) * NR, (s + 1) * NR
                        out_dma[(s // 3) % 2].dma_start(y_d[:, lo:hi],
                                                        y_sb[:, lo:hi])

    _split_multiwaits(nc)
    return nc


# ------------------------------------------------------------------ host side
def _prep_inputs(inputs: dict[str, np.ndarray]):
    """Split the full inputs into per-core in_maps for the two phases
    (index bookkeeping and value re-layout only; all arithmetic on device)."""
    q = np.asarray(inputs["q"], np.float32).reshape(N, S, CIN)
    k = np.asarray(inputs["k"], np.float32).reshape(N, S, CIN)
    v = np.asarray(inputs["v"], np.float32).reshape(N, S, CIN)
    # host pre-transpose to [i, (s, m)] and cast to bf16; vT mt-major
    qT = np.ascontiguousarray(q.transpose(2, 1, 0).reshape(CIN, S * N)).astype(NP_BF16)
    kT = np.ascontiguousarray(k.transpose(2, 1, 0).reshape(CIN, S * N)).astype(NP_BF16)
    vT = np.ascontiguousarray(
        v.reshape(NT, 128, S, CIN).transpose(3, 0, 2, 1).reshape(CIN, S * N)
    ).astype(NP_BF16)
    env = np.asarray(inputs["envelope"], np.float32)
    attn_bias = np.asarray(inputs["attn_bias"], np.float32)
    a_idx = np.asarray(inputs["atom_index"]).astype(np.int64)
    b_idx = np.asarray(inputs["batch_index"]).astype(np.int64)
    e_map = np.asarray(inputs["edge_map_tab"]).astype(np.int64)
    Wq = np.asarray(inputs["Wq"], np.float32)
    Wk = np.asarray(inputs["Wk"], np.float32)
    Wv = np.asarray(inputs["Wv"], np.float32)
    bq = np.asarray(inputs["bq"], np.float32)
    bk = np.asarray(inputs["bk"], np.float32)
    bv = np.asarray(inputs["bv"], np.float32)
    gamma = np.asarray(inputs["gamma"], np.float32)
    beta = np.asarray(inputs["beta"], np.float32)
    Wo = np.asarray(inputs["Wo"], np.float32)
    bo = np.asarray(inputs["bo"], np.float32)

    # ---- slot layout for the (atom, segment) cells
    cell = a_idx * NSEG + b_idx                      # [E]
    order = np.argsort(cell, kind="stable")
    cell_s = cell[order]
    counts = np.bincount(cell_s, minlength=N * NSEG)
    L2 = int(counts.max())
    starts = np.zeros(N * NSEG, np.int64)
    starts[1:] = np.cumsum(counts)[:-1]
    rank = np.arange(E) - starts[cell_s]             # rank within cell
    m_s = cell_s // NSEG
    g_s = cell_s % NSEG
    p_s = m_s % 128
    t_s = m_s // 128
    col = (t_s * NSEG + g_s) * L2 + rank             # free-dim position
    Wd = NT * NSEG * L2
    env_e = env[e_map]                               # value gather (re-layout)
    envS = np.zeros((128, Wd), np.float32)
    envS[p_s, col] = env_e[order]
    bS_all = []
    for h in range(H):
        bs = np.zeros((128, Wd), np.float32)
        bs[p_s, col] = attn_bias[h, e_map][order]
        bS_all.append(bs)

    # ---- per-head weight slices, expanded per spherical component, [i,(s,o)]
    WqE = Wq[L_OF_M]                                 # [9, CIN, CH]
    WkE = Wk[L_OF_M]
    WvE = Wv[L_OF_M]

    in_maps1 = []
    for h in range(H):
        sl = slice(h * D, (h + 1) * D)
        in_maps1.append({
            "qT": qT, "kT": kT, "vT": vT,
            "wq": np.ascontiguousarray(
                WqE[:, :, sl].transpose(1, 0, 2).reshape(CIN, S * D)).astype(NP_BF16),
            "wk": np.ascontiguousarray(
                WkE[:, :, sl].transpose(1, 0, 2).reshape(CIN, S * D)).astype(NP_BF16),
            "wv": np.ascontiguousarray(
                WvE[:, :, sl].transpose(1, 0, 2).reshape(CIN, S * D)).astype(NP_BF16),
            "bqkv": np.ascontiguousarray(
                np.stack([bq[sl], bk[sl], bv[sl]], axis=1)
            ),
            "bvrow": np.ascontiguousarray(bv[sl].reshape(1, D)),
            "envs": envS,
            "bs": bS_all[h],
        })

    # ---- phase-2 constants
    # gcol[p, 2l+c] = gamma[l, c*128+p];  bcol[p, c] = beta[c*128+p]
    gcol = np.zeros((128, 2 * (LMAX + 1)), np.float32)
    for l in range(LMAX + 1):
        for c in range(2):
            gcol[:, 2 * l + c] = gamma[l, c * 128:(c + 1) * 128]
    bcol = np.stack([beta[0:128], beta[128:256]], axis=1).astype(np.float32)
    woe = np.zeros((2, 128, (LMAX + 1) * CIN), NP_BF16)
    for c in range(2):
        woe[c] = Wo[:, c * 128:(c + 1) * 128, :].transpose(1, 0, 2).reshape(
            128, (LMAX + 1) * CIN).astype(NP_BF16)
    p2_const = {"gcol": gcol, "bcol": bcol, "woe": woe,
                "bo": np.ascontiguousarray(bo.reshape(CIN, 1))}
    return in_maps1, L2, p2_const


def _reorder_ao(ao_all: list[np.ndarray]) -> list[np.ndarray]:
    """[h][N, (s,d)] -> per-core [64, (s, h*D+d)] slices (pure data movement)."""
    full = np.stack([np.asarray(a).reshape(N, S, D) for a in ao_all], axis=2)
    full = full.reshape(N, S * CH)                                # [N, (S, H*D)]
    return [np.ascontiguousarray(full[c * NR:(c + 1) * NR]).astype(NP_BF16)
            for c in range(H)]


_BUILD_CACHE: dict = {}


def kernel(**inputs) -> np.ndarray:
    in_maps1, L2, p2_const = _prep_inputs(inputs)
    nc1 = _BUILD_CACHE.get(("p1", L2))
    if nc1 is None:
        nc1 = build_bass(L2)
        _BUILD_CACHE[("p1", L2)] = nc1
    res1 = run_bass_kernel_spmd(nc1, in_maps1, core_ids=list(range(H)))
    lnin_slices = _reorder_ao([r["ao"] for r in res1.results])

    nc2 = _BUILD_CACHE.get("p2")
    if nc2 is None:
        nc2 = build_phase2()
        _BUILD_CACHE["p2"] = nc2
    in_maps2 = [{"lnin": lnin_slices[c], **p2_const} for c in range(H)]
    res2 = run_bass_kernel_spmd(nc2, in_maps2, core_ids=list(range(H)))
    # yT [ci, (s, n_local)] per core -> y [N, S, CIN]
    y = np.zeros((N, S, CIN), np.float32)
    for c in range(H):
        yt = res2.results[c]["yT"].reshape(CIN, S, NR)
        y[c * NR:(c + 1) * NR] = yt.transpose(2, 1, 0)
    return np.ascontiguousarray(y)


# revision 59
# speedup vs baseline: 2.1688x; 1.0847x over previous
"""Equivariant attention (gnn_message_passing) on 8 Trainium2 NeuronCores.

Strategy (head-sharded tensor parallel, core c owns head c):

The reference materializes [H, N, E] scores/attn over E=8192 edges. Here the
edge dimension is collapsed onto the N=512 atoms at projection level:

  scores[h, n, e]   = sf[h, n, a_e] + bias[h, edge_map[e]]     (a_e = atom_index)
  attn-softmax per (batch-segment, n) then  out = attn @ vh_edges

factors exactly into dense [N, N] algebra with two tiny per-(segment, atom)
tables (NSEG=16 x N=512):

  D[g, m] = sum_{e in seg g, a_e = m} env_e   * exp(b_e)
  C[g, m] = sum_{e in seg g, a_e = m} env_e^2 * exp(b_e)
  den[g, n]  = sum_m exp(sf[m, n]) * D[g, m]           (one matmul)
  Aagg[m, n] = exp(sf[m, n]) * sum_g C[g, m] / den[g, n]
  out[n, f]  = Aagg^T @ vh[m, f]                        (one matmul)

The running-max subtraction in the reference softmax cancels exactly (up to a
+1e-16 epsilon whose relative effect is ~1e-16) and |scale*sf + b| < 20, so
unnormalized exp is safe in f32.

D/C are built on-device from "slot tensors": host packs per-(atom, segment)
edge lists into a fixed-width [128, 4*16*L2] layout (env & bias values; pads
have env=0 so they vanish), and a single free-axis reduce per table produces
it. Only integer index bookkeeping and value re-layout happen on host.

q/k/v arrive HOST-PRE-TRANSPOSED as qT/kT/vT [CIN, S*N] (channel-major), so
the kernel needs no on-device input transposes: projections read qT slices
directly.  DMAs are issued in first-use order so the PE starts ~1.5us in.

Phase 2 (per-core 64-atom slice): LN + output projection, with the output
projection done in yT [ci, (s, n)] orientation (gamma/beta folded into the
post-transpose PSUM copy); host un-transposes for free.
"""

import os
import numpy as np

import concourse.bass as bass
import concourse.tile as tile
from concourse import mybir
from concourse.bass_utils import run_bass_kernel_spmd
from concourse.masks import make_identity

# ---------------------------------------------------------------- constants
H, LMAX, NSEG = 8, 2, 16
S = (LMAX + 1) ** 2          # 9 spherical components
N, E, CIN, CH = 512, 8192, 128, 256
D = CH // H                  # 32 per-head channels
F = S * D                    # 288 per-head feature width
NT = N // 128                # 4 atom tiles
NR = N // H                  # 64 atoms per core in the LN/out stage
EPS = 1e-7
SCALE = float(np.sqrt(D / 3.0) / D)
L_OF_M = np.floor(np.sqrt(np.arange(S))).astype(np.int64)
F32 = mybir.dt.float32
F32R = mybir.dt.float32r
BF16 = mybir.dt.bfloat16
AF = mybir.ActivationFunctionType
ALU = mybir.AluOpType

import ml_dtypes
NP_BF16 = ml_dtypes.bfloat16

_DBG = bool(int(os.environ.get("KBDBG", "0")))


def _split_multiwaits(nc: bass.Bass, limit: int = 1):
    """This walrus build rejects instructions carrying more than one semaphore
    wait (and Drains carrying any). Hoist excess waits onto NOPs inserted just
    before the instruction on the same engine - semantically identical."""
    for f in nc.m.functions:
        for blk in f.blocks:
            changed = False
            out = []
            for inst in blk.instructions:
                si = inst.sync_info
                waits = list(si.on_wait) if si is not None else []
                keep = 0 if inst.opcode == "Drain" else limit
                if len(waits) > keep:
                    hoist = waits[: len(waits) - keep]
                    rest = waits[len(waits) - keep:]
                    for w in hoist:
                        nop = mybir.InstNoOp(
                            name=f"{inst.name}-w{len(out)}", ins=[], outs=[]
                        )
                        nop.engine = inst.engine
                        nop.sync_info = mybir.SyncInfo(on_wait=[w], on_update=[])
                        out.append(nop)
                    inst.sync_info = mybir.SyncInfo(
                        on_wait=rest, on_update=list(si.on_update)
                    )
                    changed = True
                out.append(inst)
            if changed:
                blk.instructions = out


def build_bass(L2: int, loop_R: int | None = None) -> bass.Bass:
    """One SPMD program; per-core data (weight slices, bias slots) comes in as
    inputs. L2 = slot width per (atom, segment) cell."""
    W = NT * NSEG * L2  # slot tensor free width per partition

    nc = bass.Bass("TRN2", target_bir_lowering=False, debug=False, num_devices=H)

    # ------------------------------------------------------------- tensors
    # host-pre-transposed bf16 inputs: qT/kT [i, (s, m)]; vT mt-major
    # [i, (t, s, j)] so each quarter is a contiguous DMA
    qT_d = nc.dram_tensor("qT", [CIN, S * N], BF16, kind="ExternalInput")
    kT_d = nc.dram_tensor("kT", [CIN, S * N], BF16, kind="ExternalInput")
    vT_d = nc.dram_tensor("vT", [CIN, S * N], BF16, kind="ExternalInput")
    wq_d = nc.dram_tensor("wq", [CIN, S * D], BF16, kind="ExternalInput")  # [i,(s,o)]
    wk_d = nc.dram_tensor("wk", [CIN, S * D], BF16, kind="ExternalInput")
    wv_d = nc.dram_tensor("wv", [CIN, S * D], BF16, kind="ExternalInput")
    bqkv_d = nc.dram_tensor("bqkv", [D, 3], F32, kind="ExternalInput")
    bvrow_d = nc.dram_tensor("bvrow", [1, D], F32, kind="ExternalInput")
    envs_d = nc.dram_tensor("envs", [128, W], F32, kind="ExternalInput")
    bs_d = nc.dram_tensor("bs", [128, W], F32, kind="ExternalInput")
    ao_d = nc.dram_tensor("ao", [N, F], BF16, kind="ExternalOutput")

    with tile.TileContext(nc) as tc:
        with (
            tc.tile_pool(name="const", bufs=1) as cpool,
            tc.tile_pool(name="feat", bufs=1) as featp,
            tc.tile_pool(name="work", bufs=1) as workp,
            tc.tile_pool(name="aop", bufs=1) as aop,
            tc.tile_pool(name="ps", bufs=4, space="PSUM") as psp,
            tc.tile_pool(name="pop", bufs=1, space="PSUM") as pop,
        ):
            def ps_tile(shape):
                return psp.tile(shape, F32, tag="ps", name="ps")

            # -------- DMA issue, first-use order, spread over engines ------
            # sync queue:   wq, qT thirds, vT quarters (mt-major layout)
            # scalar queue: wk, wv, kT thirds, envs, bs
            # gpsimd queue: tiny constants
            TH = 3 * N
            wq_sb = cpool.tile([CIN, S * D], BF16, tag="wq", name="wq")
            wk_sb = cpool.tile([CIN, S * D], BF16, tag="wk", name="wk")
            wv_sb = cpool.tile([CIN, S * D], BF16, tag="wv", name="wv")
            qTc = [cpool.tile([CIN, TH], BF16, tag=f"qT{t}", name=f"qT{t}")
                   for t in range(3)]
            kTc = [cpool.tile([CIN, TH], BF16, tag=f"kT{t}", name=f"kT{t}")
                   for t in range(3)]
            # vT quarter mt holds [i, (s, m_local)] for atom tile mt
            vTq = [cpool.tile([CIN, S * 128], BF16, tag=f"vT{m}", name=f"vT{m}")
                   for m in range(NT)]
            envs_sb = cpool.tile([128, W], F32, tag="envs", name="envs")
            bs_sb = cpool.tile([128, W], F32, tag="bs", name="bs")

            def vtq_src(m):
                return vT_d[:].rearrange("i (t x) -> i t x", t=NT)[:, m, :]

            nc.sync.dma_start(wq_sb[:], wq_d[:])
            for t in range(3):
                nc.sync.dma_start(qTc[t][:], qT_d[:, t * TH:(t + 1) * TH])
            nc.sync.dma_start(vTq[0][:], vtq_src(0))
            nc.sync.dma_start(vTq[1][:], vtq_src(1))
            nc.scalar.dma_start(wk_sb[:], wk_d[:])
            for t in range(3):
                nc.scalar.dma_start(kTc[t][:], kT_d[:, t * TH:(t + 1) * TH])
            nc.scalar.dma_start(wv_sb[:], wv_d[:])
            nc.scalar.dma_start(vTq[2][:], vtq_src(2))
            nc.scalar.dma_start(vTq[3][:], vtq_src(3))
            nc.gpsimd.dma_start(envs_sb[:], envs_d[:])
            nc.gpsimd.dma_start(bs_sb[:], bs_d[:])
            bqkv_sb = cpool.tile([D, 3], F32, tag="bqkv", name="bqkv")
            nc.gpsimd.dma_start(bqkv_sb[:], bqkv_d[:])
            bvrow_sb = cpool.tile([128, D], F32, tag="bvrow", name="bvrow")
            nc.gpsimd.dma_start(bvrow_sb[:], bvrow_d[0:1, :].to_broadcast([128, D]))
            ident = cpool.tile([128, 128], F32, tag="ident", name="ident")
            make_identity(nc, ident[:])
            eps16 = cpool.tile([1, NSEG], F32, tag="eps16", name="eps16")
            nc.gpsimd.memset(eps16[:], 1e-16)
            ones_n = cpool.tile([1, N], F32, tag="ones_n", name="ones_n")
            nc.gpsimd.memset(ones_n[:], 1.0)

            import contextlib as _ctl
            _loop = tc.For_i(0, loop_R, 1) if loop_R else _ctl.nullcontext()
            with _loop:
                copy_engines = [nc.scalar, nc.vector]
                cp_i = 0

                def copy_alt(dst_ap, src_ap):
                    nonlocal cp_i
                    eng = copy_engines[cp_i % 2]
                    cp_i += 1
                    if eng is nc.scalar:
                        eng.copy(dst_ap, src_ap)
                    else:
                        eng.tensor_copy(out=dst_ap, in_=src_ap)

                # ------- fq / fk chunk-streamed projection + psf accumulation
                # chunk layout: rows (s_local*32+o), chunks s=0..2 / 3..5 / 6..8
                # (96 rows per chunk so matmul outs land at base 0/32/64);
                # psf[mt] accumulates across chunks in 4 held banks (tags
                # shared with the po accumulators, which start strictly later)
                fq = [featp.tile([96, N], BF16, tag=f"fq{c}", name=f"fq{c}")
                      for c in range(3)]
                fk = [featp.tile([96, N], BF16, tag=f"fk{c}", name=f"fk{c}")
                      for c in range(3)]
                psf = [pop.tile([128, N], F32, tag=f"acc{mt}", name=f"psf{mt}")
                       for mt in range(NT)]
                # within chunk 0 the s components sit in row order (1, 2, 0)
                # so the biased s=0 rows are 64-aligned for the PSUM read
                # (fk uses the same permutation, so scores are unchanged)
                ROWOF = {0: 2, 1: 0, 2: 1}
                for chunk in range(3):
                    for t_c, w_sb, f_dst, t_idx in ((qTc, wq_sb, fq, 0),
                                                    (kTc, wk_sb, fk, 1)):
                        pp = ps_tile([96, N])
                        for j in range(3):
                            s = chunk * 3 + j
                            r = ROWOF[j] if chunk == 0 else j
                            nc.tensor.matmul(
                                pp[r * D:(r + 1) * D, :],
                                lhsT=w_sb[:, s * D:(s + 1) * D],
                                rhs=t_c[chunk][:, j * N:(j + 1) * N],
                                start=True, stop=True,
                            )
                        if chunk == 0:
                            # bias on s=0 rows (l=0 invariant component)
                            copy_alt(f_dst[0][0:64, :], pp[0:64, :])
                            nc.vector.tensor_scalar_add(
                                f_dst[0][64:96, :], pp[64:96, :],
                                bqkv_sb[:, t_idx:t_idx + 1])
                        else:
                            copy_alt(f_dst[chunk][:], pp[:])
                    for mt in range(NT):
                        nc.tensor.matmul(
                            psf[mt][:],
                            lhsT=fk[chunk][:, mt * 128:(mt + 1) * 128],
                            rhs=fq[chunk][:],
                            start=(chunk == 0), stop=(chunk == 2),
                            skip_group_check=True,
                        )

                # ------------------------------ D / C tables (early, off-path)
                ebs = workp.tile([128, W], F32, tag="ebs", name="ebs")
                nc.scalar.activation(ebs[:], bs_sb[:], AF.Exp)
                wD = workp.tile([128, W], F32, tag="wD", name="wD")
                nc.gpsimd.tensor_tensor(out=wD[:], in0=envs_sb[:], in1=ebs[:], op=ALU.mult)
                wC = workp.tile([128, W], F32, tag="wC", name="wC")
                nc.gpsimd.tensor_tensor(out=wC[:], in0=wD[:], in1=envs_sb[:], op=ALU.mult)
                d_t = featp.tile([128, NT * NSEG], F32, tag="d_t", name="d_t")  # [m_p, (mt, g)]
                c_t = featp.tile([128, NT * NSEG], F32, tag="c_t", name="c_t")
                with nc.allow_low_precision(reason="f32r is 32-bit storage"):
                    nc.vector.reduce_sum(
                        out=d_t[:].rearrange("p (t g) -> p t g", t=NT).bitcast(F32R),
                        in_=wD[:].rearrange("p (t g j) -> p t g j", t=NT, g=NSEG),
                        axis=mybir.AxisListType.X,
                    )
                nc.vector.reduce_sum(
                    out=c_t[:].rearrange("p (t g) -> p t g", t=NT),
                    in_=wC[:].rearrange("p (t g j) -> p t g j", t=NT, g=NSEG),
                    axis=mybir.AxisListType.X,
                )
                # C transposed to [g, m]
                c_sb = featp.tile([NSEG, N], F32, tag="c_sb", name="c_sb")
                for mt in range(NT):
                    pc = ps_tile([NSEG, 128])
                    nc.tensor.transpose(
                        pc[:], c_t[:, mt * NSEG:(mt + 1) * NSEG], ident[:]
                    )
                    nc.scalar.copy(c_sb[:, mt * 128:(mt + 1) * 128].bitcast(F32R), pc[:])

                # -------------------------------- vhn [m, (s,o)] per m-tile
                vhn = [featp.tile([128, F], BF16, tag=f"vhn{mt}", name=f"vhn{mt}") for mt in range(NT)]
                for mt in range(NT):
                    pv = ps_tile([128, F])
                    for s in range(S):
                        nc.tensor.matmul(
                            pv[:, s * D:(s + 1) * D],
                            lhsT=vTq[mt][:, s * 128:(s + 1) * 128],
                            rhs=wv_sb[:, s * D:(s + 1) * D],
                            start=True, stop=True,
                        )
                    nc.vector.tensor_copy(out=vhn[mt][:, D:F], in_=pv[:, D:F])
                    nc.vector.tensor_tensor(
                        out=vhn[mt][:, 0:D],
                        in0=pv[:, 0:D], in1=bvrow_sb[:], op=ALU.add,
                    )

                # ---------------- exp + denominator accumulation, staggered
                exp_sf = [featp.tile([128, N], F32, tag=f"esf{mt}", name=f"esf{mt}") for mt in range(NT)]
                pden = ps_tile([NSEG, N])
                nc.tensor.matmul(
                    pden[:], lhsT=eps16[:].bitcast(F32R),
                    rhs=ones_n[:].bitcast(F32R), start=True, stop=False,
                    skip_group_check=True,
                )
                for mt in range(NT):
                    nc.scalar.activation(exp_sf[mt][:].bitcast(F32R), psf[mt][:],
                                         AF.Exp, scale=SCALE)
                    nc.tensor.matmul(
                        pden[:], lhsT=d_t[:, mt * NSEG:(mt + 1) * NSEG].bitcast(F32R),
                        rhs=exp_sf[mt][:].bitcast(F32R),
                        start=False, stop=(mt == NT - 1),
                        skip_group_check=True,
                    )
                dd = featp.tile([NSEG, N], F32, tag="dd", name="dd")
                with nc.allow_low_precision(reason="f32r is 32-bit storage"):
                    nc.vector.reciprocal(dd[:].bitcast(F32R), pden[:])

                # ------- per m-tile: aggt = exp_sf * (C^T dd); vhn; att-out
                # po[nt] accumulates mt-major in the acc banks (freed by exp)
                aggt = [featp.tile([128, N], BF16, tag=f"aggt{mt}", name=f"aggt{mt}") for mt in range(NT)]
                po = [pop.tile([128, F], F32, tag=f"acc{nt}", name=f"po{nt}")
                      for nt in range(NT)]
                agg_engines = [nc.vector, nc.vector]
                for mt in range(NT):
                    pT = ps_tile([128, N])
                    nc.tensor.matmul(
                        pT[:], lhsT=c_sb[:, mt * 128:(mt + 1) * 128].bitcast(F32R),
                        rhs=dd[:].bitcast(F32R),
                        start=True, stop=True,
                    )
                    agg_engines[mt % 2].tensor_tensor(
                        out=aggt[mt][:], in0=exp_sf[mt][:],
                        in1=pT[:], op=ALU.mult)
                    for nt in range(NT):
                        nc.tensor.matmul(
                            po[nt][:],
                            lhsT=aggt[mt][:, nt * 128:(nt + 1) * 128],
                            rhs=vhn[mt][:],
                            start=(mt == 0), stop=(mt == NT - 1),
                            skip_group_check=True,
                        )
                ao = aop.tile([128, NT * F], BF16, tag="ao", name="ao")
                for nt in range(NT):
                    copy_alt(ao[:, nt * F:(nt + 1) * F], po[nt][:])
                nc.sync.dma_start(
                    ao_d[:].rearrange("(t p) f -> p t f", t=NT),
                    ao[:].rearrange("p (t f) -> p t f", t=NT))

    _split_multiwaits(nc)
    return nc


def build_phase2(loop_R: int | None = None) -> bass.Bass:
    """Equivariant layernorm + output projection on a 64-atom slice.
    Input lnin [64, (s, ch)]; output yT [ci, (s, n)] (host un-transposes).
    gamma/beta are folded into the post-transpose PSUM->SBUF copies as
    per-partition tensor_scalar ops; same NEFF on all cores."""
    nc = bass.Bass("TRN2", target_bir_lowering=False, debug=False, num_devices=H)
    lnin_d = nc.dram_tensor("lnin", [NR, S * CH], BF16, kind="ExternalInput")
    gcol_d = nc.dram_tensor("gcol", [128, 2 * (LMAX + 1)], F32, kind="ExternalInput")
    bcol_d = nc.dram_tensor("bcol", [128, 2], BF16, kind="ExternalInput")
    # compact per-l output weights: [c_half, i, (l, ci)]
    woe_d = nc.dram_tensor("woe", [2, 128, (LMAX + 1) * CIN], BF16, kind="ExternalInput")
    bo_d = nc.dram_tensor("bo", [CIN, 1], F32, kind="ExternalInput")
    y_d = nc.dram_tensor("yT", [CIN, S * NR], F32, kind="ExternalOutput")

    with tile.TileContext(nc) as tc:
        with (
            tc.tile_pool(name="const", bufs=1) as cpool,
            tc.tile_pool(name="work", bufs=1) as workp,
            tc.tile_pool(name="tp", bufs=4) as tpp,
            tc.tile_pool(name="ps", bufs=1, space="PSUM") as psp,
            tc.tile_pool(name="plbp", bufs=4, space="PSUM") as plbp,
            tc.tile_pool(name="pyg", bufs=3, space="PSUM") as pygp,
        ):
            def ps_tile(shape):
                return psp.tile(shape, F32, tag="ps", name="ps")

            lnin = workp.tile([NR, S * CH], BF16, tag="lnin", name="lnin")
            # section DMAs so the l=0 chain starts early
            nc.sync.dma_start(lnin[:, 0:CH], lnin_d[:, 0:CH])
            nc.sync.dma_start(lnin[:, 4 * CH:S * CH], lnin_d[:, 4 * CH:S * CH])
            nc.sync.dma_start(lnin[:, CH:4 * CH], lnin_d[:, CH:4 * CH])
            woe_sb = [
                cpool.tile([128, (LMAX + 1) * CIN], BF16, tag=f"woe{c}", name=f"woe{c}")
                for c in range(2)
            ]
            gcol_sb = cpool.tile([128, 2 * (LMAX + 1)], F32, tag="gcol", name="gcol")
            bcol_sb = cpool.tile([128, 2], BF16, tag="bcol", name="bcol")
            bo_sb = cpool.tile([CIN, 1], F32, tag="bo", name="bo")
            for c in range(2):
                nc.sync.dma_start(woe_sb[c][:], woe_d[c, :, :])
            nc.sync.dma_start(gcol_sb[:], gcol_d[:])
            nc.sync.dma_start(bcol_sb[:], bcol_d[:])
            nc.sync.dma_start(bo_sb[:], bo_d[:])
            ident = cpool.tile([128, 128], BF16, tag="ident", name="ident")
            make_identity(nc, ident[:])
            eps_sb = cpool.tile([128, 1], F32, tag="epsc", name="epsc")
            nc.gpsimd.memset(eps_sb[:], EPS)

            ones_r = cpool.tile([NR, 128], BF16, tag="ones_r", name="ones_r")
            nc.gpsimd.memset(ones_r[:], 1.0)

            import contextlib as _ctl
            _loop = tc.For_i(0, loop_R, 1) if loop_R else _ctl.nullcontext()
            with _loop:
                # ---- LN statistics (per-atom scalars)
                # l=0: one-pass LN over CH: var = E[x^2] - mu^2
                x0 = lnin[:, 0:CH]
                sc0 = workp.tile([NR, CH], F32, tag="sc0", name="sc0")
                mu = workp.tile([NR, 1], F32, tag="mu", name="mu")
                nc.scalar.activation(sc0[:], x0, AF.Copy, scale=1.0 / CH,
                                     accum_out=mu[:])
                sq0 = workp.tile([NR, CH], F32, tag="sq0", name="sq0")
                vs = workp.tile([NR, 1], F32, tag="vs", name="vs")
                nc.vector.tensor_tensor(out=sq0[:], in0=x0, in1=x0, op=ALU.mult)
                nc.vector.reduce_sum(out=vs[:], in_=sq0[:],
                                     axis=mybir.AxisListType.X)
                mu2 = workp.tile([NR, 1], F32, tag="mu2", name="mu2")
                nc.gpsimd.tensor_tensor(out=mu2[:], in0=mu[:], in1=mu[:], op=ALU.mult)
                ebias = workp.tile([NR, 1], F32, tag="ebias", name="ebias")
                nc.gpsimd.tensor_scalar(out=ebias[:], in0=mu2[:], scalar1=-1.0,
                                        scalar2=EPS, op0=ALU.mult, op1=ALU.add)
                sd = workp.tile([NR, 1], F32, tag="sd", name="sd")
                nc.scalar.activation(sd[:], vs[:], AF.Sqrt, scale=1.0 / CH,
                                     bias=ebias[:, 0:1])
                rstd = workp.tile([NR, 1], F32, tag="rstd", name="rstd")
                nc.vector.reciprocal(rstd[:], sd[:])
                mr = workp.tile([NR, 1], F32, tag="mr", name="mr")
                nc.gpsimd.tensor_tensor(out=mr[:], in0=mu[:], in1=rstd[:], op=ALU.mult)
                nmr = workp.tile([NR, 1], F32, tag="nmr", name="nmr")
                nc.gpsimd.tensor_scalar_mul(nmr[:], mr[:], -1.0)
                # l=2 then l=1 RMS stats (Act Square+accum; recips on DVE)
                rr = {}
                for l in (2, 1):
                    lo, hi = (l * l) * CH, ((l + 1) * (l + 1)) * CH
                    width = hi - lo
                    ms = workp.tile([NR, 1], F32, tag=f"ms{l}", name=f"ms{l}")
                    sql = workp.tile([NR, width], F32, tag=f"sq{l}", name=f"sq{l}")
                    nc.scalar.activation(sql[:], lnin[:, lo:hi], AF.Square,
                                         accum_out=ms[:])
                    sdl = workp.tile([NR, 1], F32, tag=f"sd{l}", name=f"sd{l}")
                    nc.scalar.activation(sdl[:], ms[:], AF.Sqrt, scale=1.0 / width,
                                         bias=eps_sb[0:NR, 0:1])
                    rrl = workp.tile([NR, 1], F32, tag=f"rr{l}", name=f"rr{l}")
                    nc.vector.reciprocal(rrl[:], sdl[:])
                    rr[l] = rrl

                # ---- gamma folded into weights (DVE bf16 fast mode)
                woe_g = [workp.tile([128, (LMAX + 1) * CIN], BF16, tag=f"wg{c}",
                                    name=f"wg{c}") for c in range(2)]
                for c in range(2):
                    for l in range(LMAX + 1):
                        nc.vector.tensor_scalar_mul(
                            woe_g[c][:, l * CIN:(l + 1) * CIN],
                            woe_sb[c][:, l * CIN:(l + 1) * CIN],
                            gcol_sb[:, 2 * l + c:2 * l + c + 1])
                # beta contribution: pbw[ci] = sum_ch beta[ch] * Wo[0][ch, ci]
                pbw = ps_tile([CIN, 1])
                for c in range(2):
                    nc.tensor.matmul(
                        pbw[:], lhsT=woe_sb[c][:, 0:CIN],
                        rhs=bcol_sb[:, c:c + 1],
                        start=(c == 0), stop=(c == 1))
                bo0 = workp.tile([CIN, 1], F32, tag="bo0", name="bo0")
                nc.vector.tensor_tensor(out=bo0[:], in0=pbw[:], in1=bo_sb[:],
                                        op=ALU.add)

                # ---- diag(scale) tiles: the transpose matmul applies the
                # per-atom LN scaling for free (rhs = diag instead of I)
                diag = {}
                for l, scl in ((2, rr[2]), (0, rstd), (1, rr[1])):
                    dg = workp.tile([NR, NR], BF16, tag=f"diag{l}", name=f"diag{l}")
                    nc.gpsimd.tensor_scalar_mul(dg[:], ident[0:NR, 0:NR],
                                                scl[:, 0:1])
                    diag[l] = dg
                dnm = workp.tile([NR, NR], BF16, tag="dnm", name="dnm")
                nc.gpsimd.tensor_scalar_mul(dnm[:], ident[0:NR, 0:NR], nmr[:, 0:1])

                # ---- per s (l=2 block first): scale+transpose -> lnT -> yT
                y_sb = workp.tile([CIN, S * NR], F32, tag="ysb", name="ysb")
                out_dma = [nc.sync, nc.scalar]
                for si, s in enumerate([4, 5, 6, 7, 8, 0, 1, 2, 3]):
                    l = int(L_OF_M[s])
                    pl = plbp.tile([128, 2 * NR], F32, tag="plb", name="plb")
                    for c in range(2):
                        nc.tensor.matmul(
                            pl[:, c * NR:(c + 1) * NR],
                            lhsT=lnin[:, s * CH + c * 128: s * CH + (c + 1) * 128],
                            rhs=diag[l][:],
                            start=True, stop=(s > 0),
                            skip_group_check=True,
                        )
                        if s == 0:
                            nc.tensor.matmul(
                                pl[:, c * NR:(c + 1) * NR],
                                lhsT=ones_r[:],
                                rhs=dnm[:],
                                start=False, stop=True,
                                skip_group_check=True,
                            )
                    lnT = tpp.tile([128, 2 * NR], BF16, tag="lnT", name="lnT")
                    if si % 2:
                        nc.vector.tensor_copy(out=lnT[:], in_=pl[:])
                    else:
                        nc.scalar.copy(lnT[:], pl[:])
                    py = pygp.tile([CIN, NR], F32, tag="pyg", name="pyg")
                    for c in range(2):
                        nc.tensor.matmul(
                            py[:],
                            lhsT=woe_g[c][:, l * CIN:(l + 1) * CIN],
                            rhs=lnT[:, c * NR:(c + 1) * NR],
                            start=(c == 0), stop=(c == 1),
                        )
                    bias = bo0[:, 0:1] if s == 0 else bo_sb[:, 0:1]
                    if si % 2:
                        nc.vector.tensor_scalar_add(y_sb[:, s * NR:(s + 1) * NR],
                                                    py[:], bias)
                    else:
                        nc.scalar.activation(y_sb[:, s * NR:(s + 1) * NR], py[:],
                                             AF.Identity, bias=bias)
                    if s == 8:
                        out_dma[0].dma_start(y_d[:, 4 * NR:S * NR],
                                             y_sb[:, 4 * NR:S * NR])
                    elif s == 3:
                        out_dma[1].dma_start(y_d[:, 0:4 * NR],
                                             y_sb[:, 0:4 * NR])

    _split_multiwaits(nc)
    return nc


# ------------------------------------------------------------------ host side
def _prep_inputs(inputs: dict[str, np.ndarray]):
    """Split the full inputs into per-core in_maps for the two phases
    (index bookkeeping and value re-layout only; all arithmetic on device)."""
    q = np.asarray(inputs["q"], np.float32).reshape(N, S, CIN)
    k = np.asarray(inputs["k"], np.float32).reshape(N, S, CIN)
    v = np.asarray(inputs["v"], np.float32).reshape(N, S, CIN)
    # host pre-transpose to [i, (s, m)] and cast to bf16; vT mt-major
    qT = np.ascontiguousarray(q.transpose(2, 1, 0).reshape(CIN, S * N)).astype(NP_BF16)
    kT = np.ascontiguousarray(k.transpose(2, 1, 0).reshape(CIN, S * N)).astype(NP_BF16)
    vT = np.ascontiguousarray(
        v.reshape(NT, 128, S, CIN).transpose(3, 0, 2, 1).reshape(CIN, S * N)
    ).astype(NP_BF16)
    env = np.asarray(inputs["envelope"], np.float32)
    attn_bias = np.asarray(inputs["attn_bias"], np.float32)
    a_idx = np.asarray(inputs["atom_index"]).astype(np.int64)
    b_idx = np.asarray(inputs["batch_index"]).astype(np.int64)
    e_map = np.asarray(inputs["edge_map_tab"]).astype(np.int64)
    Wq = np.asarray(inputs["Wq"], np.float32)
    Wk = np.asarray(inputs["Wk"], np.float32)
    Wv = np.asarray(inputs["Wv"], np.float32)
    bq = np.asarray(inputs["bq"], np.float32)
    bk = np.asarray(inputs["bk"], np.float32)
    bv = np.asarray(inputs["bv"], np.float32)
    gamma = np.asarray(inputs["gamma"], np.float32)
    beta = np.asarray(inputs["beta"], np.float32)
    Wo = np.asarray(inputs["Wo"], np.float32)
    bo = np.asarray(inputs["bo"], np.float32)

    # ---- slot layout for the (atom, segment) cells
    cell = a_idx * NSEG + b_idx                      # [E]
    order = np.argsort(cell, kind="stable")
    cell_s = cell[order]
    counts = np.bincount(cell_s, minlength=N * NSEG)
    L2 = int(counts.max())
    starts = np.zeros(N * NSEG, np.int64)
    starts[1:] = np.cumsum(counts)[:-1]
    rank = np.arange(E) - starts[cell_s]             # rank within cell
    m_s = cell_s // NSEG
    g_s = cell_s % NSEG
    p_s = m_s % 128
    t_s = m_s // 128
    col = (t_s * NSEG + g_s) * L2 + rank             # free-dim position
    Wd = NT * NSEG * L2
    env_e = env[e_map]                               # value gather (re-layout)
    envS = np.zeros((128, Wd), np.float32)
    envS[p_s, col] = env_e[order]
    bS_all = []
    for h in range(H):
        bs = np.zeros((128, Wd), np.float32)
        bs[p_s, col] = attn_bias[h, e_map][order]
        bS_all.append(bs)

    # ---- per-head weight slices, expanded per spherical component, [i,(s,o)]
    WqE = Wq[L_OF_M]                                 # [9, CIN, CH]
    WkE = Wk[L_OF_M]
    WvE = Wv[L_OF_M]

    in_maps1 = []
    for h in range(H):
        sl = slice(h * D, (h + 1) * D)
        in_maps1.append({
            "qT": qT, "kT": kT, "vT": vT,
            "wq": np.ascontiguousarray(
                WqE[:, :, sl].transpose(1, 0, 2).reshape(CIN, S * D)).astype(NP_BF16),
            "wk": np.ascontiguousarray(
                WkE[:, :, sl].transpose(1, 0, 2).reshape(CIN, S * D)).astype(NP_BF16),
            "wv": np.ascontiguousarray(
                WvE[:, :, sl].transpose(1, 0, 2).reshape(CIN, S * D)).astype(NP_BF16),
            "bqkv": np.ascontiguousarray(
                np.stack([bq[sl], bk[sl], bv[sl]], axis=1)
            ),
            "bvrow": np.ascontiguousarray(bv[sl].reshape(1, D)),
            "envs": envS,
            "bs": bS_all[h],
        })

    # ---- phase-2 constants
    # gcol[p, 2l+c] = gamma[l, c*128+p];  bcol[p, c] = beta[c*128+p]
    gcol = np.zeros((128, 2 * (LMAX + 1)), np.float32)
    for l in range(LMAX + 1):
        for c in range(2):
            gcol[:, 2 * l + c] = gamma[l, c * 128:(c + 1) * 128]
    bcol = np.stack([beta[0:128], beta[128:256]], axis=1).astype(NP_BF16)
    woe = np.zeros((2, 128, (LMAX + 1) * CIN), NP_BF16)
    for c in range(2):
        woe[c] = Wo[:, c * 128:(c + 1) * 128, :].transpose(1, 0, 2).reshape(
            128, (LMAX + 1) * CIN).astype(NP_BF16)
    p2_const = {"gcol": gcol, "bcol": bcol, "woe": woe,
                "bo": np.ascontiguousarray(bo.reshape(CIN, 1))}
    return in_maps1, L2, p2_const


def _reorder_ao(ao_all: list[np.ndarray]) -> list[np.ndarray]:
    """[h][N, (s,d)] -> per-core [64, (s, h*D+d)] slices (pure data movement)."""
    full = np.stack([np.asarray(a).reshape(N, S, D) for a in ao_all], axis=2)
    full = full.reshape(N, S * CH)                                # [N, (S, H*D)]
    return [np.ascontiguousarray(full[c * NR:(c + 1) * NR]).astype(NP_BF16)
            for c in range(H)]


_BUILD_CACHE: dict = {}


def kernel(**inputs) -> np.ndarray:
    in_maps1, L2, p2_const = _prep_inputs(inputs)
    nc1 = _BUILD_CACHE.get(("p1", L2))
    if nc1 is None:
        nc1 = build_bass(L2)
        _BUILD_CACHE[("p1", L2)] = nc1
    res1 = run_bass_kernel_spmd(nc1, in_maps1, core_ids=list(range(H)))
    lnin_slices = _reorder_ao([r["ao"] for r in res1.results])

    nc2 = _BUILD_CACHE.get("p2")
    if nc2 is None:
        nc2 = build_phase2()
        _BUILD_CACHE["p2"] = nc2
    in_maps2 = [{"lnin": lnin_slices[c], **p2_const} for c in range(H)]
    res2 = run_bass_kernel_spmd(nc2, in_maps2, core_ids=list(range(H)))
    # yT [ci, (s, n_local)] per core -> y [N, S, CIN]
    y = np.zeros((N, S, CIN), np.float32)
    for c in range(H):
        yt = res2.results[c]["yT"].reshape(CIN, S, NR)
        y[c * NR:(c + 1) * NR] = yt.transpose(2, 1, 0)
    return np.ascontiguousarray(y)


# revision 70
# speedup vs baseline: 2.1769x; 1.0037x over previous
"""Equivariant attention (gnn_message_passing) on 8 Trainium2 NeuronCores.

Strategy (head-sharded tensor parallel, core c owns head c):

The reference materializes [H, N, E] scores/attn over E=8192 edges. Here the
edge dimension is collapsed onto the N=512 atoms at projection level:

  scores[h, n, e]   = sf[h, n, a_e] + bias[h, edge_map[e]]     (a_e = atom_index)
  attn-softmax per (batch-segment, n) then  out = attn @ vh_edges

factors exactly into dense [N, N] algebra with two tiny per-(segment, atom)
tables (NSEG=16 x N=512):

  D[g, m] = sum_{e in seg g, a_e = m} env_e   * exp(b_e)
  C[g, m] = sum_{e in seg g, a_e = m} env_e^2 * exp(b_e)
  den[g, n]  = sum_m exp(sf[m, n]) * D[g, m]           (one matmul)
  Aagg[m, n] = exp(sf[m, n]) * sum_g C[g, m] / den[g, n]
  out[n, f]  = Aagg^T @ vh[m, f]                        (one matmul)

The running-max subtraction in the reference softmax cancels exactly (up to a
+1e-16 epsilon whose relative effect is ~1e-16) and |scale*sf + b| < 20, so
unnormalized exp is safe in f32.

D/C are built on-device from "slot tensors": host packs per-(atom, segment)
edge lists into a fixed-width [128, 4*16*L2] layout (env & bias values; pads
have env=0 so they vanish), and a single free-axis reduce per table produces
it. Only integer index bookkeeping and value re-layout happen on host.

q/k/v arrive HOST-PRE-TRANSPOSED as qT/kT/vT [CIN, S*N] (channel-major), so
the kernel needs no on-device input transposes: projections read qT slices
directly.  DMAs are issued in first-use order so the PE starts ~1.5us in.

Phase 2 (per-core 64-atom slice): LN + output projection, with the output
projection done in yT [ci, (s, n)] orientation (gamma/beta folded into the
post-transpose PSUM copy); host un-transposes for free.
"""

import os
import numpy as np

import concourse.bass as bass
import concourse.tile as tile
from concourse import mybir
from concourse.bass_utils import run_bass_kernel_spmd
from concourse.masks import make_identity

# ---------------------------------------------------------------- constants
H, LMAX, NSEG = 8, 2, 16
S = (LMAX + 1) ** 2          # 9 spherical components
N, E, CIN, CH = 512, 8192, 128, 256
D = CH // H                  # 32 per-head channels
F = S * D                    # 288 per-head feature width
NT = N // 128                # 4 atom tiles
NR = N // H                  # 64 atoms per core in the LN/out stage
EPS = 1e-7
SCALE = float(np.sqrt(D / 3.0) / D)
L_OF_M = np.floor(np.sqrt(np.arange(S))).astype(np.int64)
F32 = mybir.dt.float32
F32R = mybir.dt.float32r
BF16 = mybir.dt.bfloat16
AF = mybir.ActivationFunctionType
ALU = mybir.AluOpType

import ml_dtypes
NP_BF16 = ml_dtypes.bfloat16

_DBG = bool(int(os.environ.get("KBDBG", "0")))


def _split_multiwaits(nc: bass.Bass, limit: int = 1):
    """This walrus build rejects instructions carrying more than one semaphore
    wait (and Drains carrying any). Hoist excess waits onto NOPs inserted just
    before the instruction on the same engine - semantically identical."""
    for f in nc.m.functions:
        for blk in f.blocks:
            changed = False
            out = []
            for inst in blk.instructions:
                si = inst.sync_info
                waits = list(si.on_wait) if si is not None else []
                keep = 0 if inst.opcode == "Drain" else limit
                if len(waits) > keep:
                    hoist = waits[: len(waits) - keep]
                    rest = waits[len(waits) - keep:]
                    for w in hoist:
                        nop = mybir.InstNoOp(
                            name=f"{inst.name}-w{len(out)}", ins=[], outs=[]
                        )
                        nop.engine = inst.engine
                        nop.sync_info = mybir.SyncInfo(on_wait=[w], on_update=[])
                        out.append(nop)
                    inst.sync_info = mybir.SyncInfo(
                        on_wait=rest, on_update=list(si.on_update)
                    )
                    changed = True
                out.append(inst)
            if changed:
                blk.instructions = out


def build_bass(L2: int, loop_R: int | None = None) -> bass.Bass:
    """One SPMD program; per-core data (weight slices, bias slots) comes in as
    inputs. L2 = slot width per (atom, segment) cell."""
    W = NT * NSEG * L2  # slot tensor free width per partition

    nc = bass.Bass("TRN2", target_bir_lowering=False, debug=False, num_devices=H)

    # ------------------------------------------------------------- tensors
    # host-pre-transposed bf16 inputs: qT/kT [i, (s, m)]; vT mt-major
    # [i, (t, s, j)] so each quarter is a contiguous DMA
    qT_d = nc.dram_tensor("qT", [CIN, S * N], BF16, kind="ExternalInput")
    kT_d = nc.dram_tensor("kT", [CIN, S * N], BF16, kind="ExternalInput")
    vT_d = nc.dram_tensor("vT", [CIN, S * N], BF16, kind="ExternalInput")
    wq_d = nc.dram_tensor("wq", [CIN, S * D], BF16, kind="ExternalInput")  # [i,(s,o)]
    wk_d = nc.dram_tensor("wk", [CIN, S * D], BF16, kind="ExternalInput")
    wv_d = nc.dram_tensor("wv", [CIN, S * D], BF16, kind="ExternalInput")
    bqkv_d = nc.dram_tensor("bqkv", [D, 3], F32, kind="ExternalInput")
    bvrow_d = nc.dram_tensor("bvrow", [1, D], F32, kind="ExternalInput")
    envs_d = nc.dram_tensor("envs", [128, W], F32, kind="ExternalInput")
    bs_d = nc.dram_tensor("bs", [128, W], F32, kind="ExternalInput")
    ao_d = nc.dram_tensor("ao", [N, F], BF16, kind="ExternalOutput")

    with tile.TileContext(nc) as tc:
        with (
            tc.tile_pool(name="const", bufs=1) as cpool,
            tc.tile_pool(name="feat", bufs=1) as featp,
            tc.tile_pool(name="work", bufs=1) as workp,
            tc.tile_pool(name="aop", bufs=1) as aop,
            tc.tile_pool(name="ps", bufs=4, space="PSUM") as psp,
            tc.tile_pool(name="pop", bufs=1, space="PSUM") as pop,
        ):
            def ps_tile(shape):
                return psp.tile(shape, F32, tag="ps", name="ps")

            # -------- DMA issue, first-use order, spread over engines ------
            # sync queue:   wq, qT thirds, vT quarters (mt-major layout)
            # scalar queue: wk, wv, kT thirds, envs, bs
            # gpsimd queue: tiny constants
            TH = 3 * N
            wq_sb = cpool.tile([CIN, S * D], BF16, tag="wq", name="wq")
            wk_sb = cpool.tile([CIN, S * D], BF16, tag="wk", name="wk")
            wv_sb = cpool.tile([CIN, S * D], BF16, tag="wv", name="wv")
            qTc = [cpool.tile([CIN, TH], BF16, tag=f"qT{t}", name=f"qT{t}")
                   for t in range(3)]
            kTc = [cpool.tile([CIN, TH], BF16, tag=f"kT{t}", name=f"kT{t}")
                   for t in range(3)]
            # vT quarter mt holds [i, (s, m_local)] for atom tile mt
            vTq = [cpool.tile([CIN, S * 128], BF16, tag=f"vT{m}", name=f"vT{m}")
                   for m in range(NT)]
            envs_sb = cpool.tile([128, W], F32, tag="envs", name="envs")
            bs_sb = cpool.tile([128, W], F32, tag="bs", name="bs")

            def vtq_src(m):
                return vT_d[:].rearrange("i (t x) -> i t x", t=NT)[:, m, :]

            nc.sync.dma_start(wq_sb[:], wq_d[:])
            for t in range(3):
                nc.sync.dma_start(qTc[t][:], qT_d[:, t * TH:(t + 1) * TH])
            nc.sync.dma_start(vTq[0][:], vtq_src(0))
            nc.sync.dma_start(vTq[1][:], vtq_src(1))
            nc.scalar.dma_start(wk_sb[:], wk_d[:])
            for t in range(3):
                nc.scalar.dma_start(kTc[t][:], kT_d[:, t * TH:(t + 1) * TH])
            nc.scalar.dma_start(wv_sb[:], wv_d[:])
            nc.gpsimd.dma_start(envs_sb[:], envs_d[:])
            nc.gpsimd.dma_start(bs_sb[:], bs_d[:])
            nc.scalar.dma_start(vTq[2][:], vtq_src(2))
            nc.scalar.dma_start(vTq[3][:], vtq_src(3))
            bqkv_sb = cpool.tile([D, 3], F32, tag="bqkv", name="bqkv")
            nc.gpsimd.dma_start(bqkv_sb[:], bqkv_d[:])
            bvrow_sb = cpool.tile([128, D], F32, tag="bvrow", name="bvrow")
            nc.gpsimd.dma_start(bvrow_sb[:], bvrow_d[0:1, :].to_broadcast([128, D]))
            ident = cpool.tile([128, 128], F32, tag="ident", name="ident")
            make_identity(nc, ident[:])
            eps16 = cpool.tile([1, NSEG], F32, tag="eps16", name="eps16")
            nc.gpsimd.memset(eps16[:], 1e-16)
            ones_n = cpool.tile([1, N], F32, tag="ones_n", name="ones_n")
            nc.gpsimd.memset(ones_n[:], 1.0)

            import contextlib as _ctl
            _loop = tc.For_i(0, loop_R, 1) if loop_R else _ctl.nullcontext()
            with _loop:
                copy_engines = [nc.scalar, nc.vector]
                cp_i = 0

                def copy_alt(dst_ap, src_ap):
                    nonlocal cp_i
                    eng = copy_engines[cp_i % 2]
                    cp_i += 1
                    if eng is nc.scalar:
                        eng.copy(dst_ap, src_ap)
                    else:
                        eng.tensor_copy(out=dst_ap, in_=src_ap)

                def copy_dve(dst_ap, src_ap):
                    nc.vector.tensor_copy(out=dst_ap, in_=src_ap)

                # ------------------------------ D / C tables (early, off-path)
                ebs = workp.tile([128, W], F32, tag="ebs", name="ebs")
                nc.scalar.activation(ebs[:], bs_sb[:], AF.Exp)
                wD = workp.tile([128, W], F32, tag="wD", name="wD")
                nc.vector.tensor_tensor(out=wD[:], in0=envs_sb[:], in1=ebs[:], op=ALU.mult)
                wC = workp.tile([128, W], F32, tag="wC", name="wC")
                nc.vector.tensor_tensor(out=wC[:], in0=wD[:], in1=envs_sb[:], op=ALU.mult)
                d_t = featp.tile([128, NT * NSEG], F32, tag="d_t", name="d_t")  # [m_p, (mt, g)]
                c_t = featp.tile([128, NT * NSEG], F32, tag="c_t", name="c_t")
                with nc.allow_low_precision(reason="f32r is 32-bit storage"):
                    nc.vector.reduce_sum(
                        out=d_t[:].rearrange("p (t g) -> p t g", t=NT).bitcast(F32R),
                        in_=wD[:].rearrange("p (t g j) -> p t g j", t=NT, g=NSEG),
                        axis=mybir.AxisListType.X,
                    )
                nc.vector.reduce_sum(
                    out=c_t[:].rearrange("p (t g) -> p t g", t=NT),
                    in_=wC[:].rearrange("p (t g j) -> p t g j", t=NT, g=NSEG),
                    axis=mybir.AxisListType.X,
                )
                # ------- fq / fk chunk-streamed projection + psf accumulation
                # chunk layout: rows (s_local*32+o), chunks s=0..2 / 3..5 / 6..8
                # (96 rows per chunk so matmul outs land at base 0/32/64);
                # psf[mt] accumulates across chunks in 4 held banks (tags
                # shared with the po accumulators, which start strictly later)
                fq = [featp.tile([96, N], BF16, tag=f"fq{c}", name=f"fq{c}")
                      for c in range(3)]
                fk = [featp.tile([96, N], BF16, tag=f"fk{c}", name=f"fk{c}")
                      for c in range(3)]
                psf = [pop.tile([128, N], F32, tag=f"acc{mt}", name=f"psf{mt}")
                       for mt in range(NT)]
                # within chunk 0 the s components sit in row order (1, 2, 0)
                # so the biased s=0 rows are 64-aligned for the PSUM read
                # (fk uses the same permutation, so scores are unchanged)
                ROWOF = {0: 2, 1: 0, 2: 1}
                for chunk in range(3):
                    for t_c, w_sb, f_dst, t_idx in ((qTc, wq_sb, fq, 0),
                                                    (kTc, wk_sb, fk, 1)):
                        pp = ps_tile([96, N])
                        for j in range(3):
                            s = chunk * 3 + j
                            r = ROWOF[j] if chunk == 0 else j
                            nc.tensor.matmul(
                                pp[r * D:(r + 1) * D, :],
                                lhsT=w_sb[:, s * D:(s + 1) * D],
                                rhs=t_c[chunk][:, j * N:(j + 1) * N],
                                start=True, stop=True,
                            )
                        if chunk == 0:
                            # bias on s=0 rows (l=0 invariant component)
                            copy_dve(f_dst[0][0:64, :], pp[0:64, :])
                            nc.vector.tensor_scalar_add(
                                f_dst[0][64:96, :], pp[64:96, :],
                                bqkv_sb[:, t_idx:t_idx + 1])
                        else:
                            copy_dve(f_dst[chunk][:], pp[:])
                    for mt in range(NT):
                        nc.tensor.matmul(
                            psf[mt][:],
                            lhsT=fk[chunk][:, mt * 128:(mt + 1) * 128],
                            rhs=fq[chunk][:],
                            start=(chunk == 0), stop=(chunk == 2),
                            skip_group_check=True,
                        )

                # C transposed to [g, m]
                c_sb = featp.tile([NSEG, N], F32, tag="c_sb", name="c_sb")
                for mt in range(NT):
                    pc = ps_tile([NSEG, 128])
                    nc.tensor.transpose(
                        pc[:], c_t[:, mt * NSEG:(mt + 1) * NSEG], ident[:]
                    )
                    nc.vector.tensor_copy(out=c_sb[:, mt * 128:(mt + 1) * 128].bitcast(F32R), in_=pc[:])

                # -------------------------------- vhn [m, (s,o)] per m-tile
                vhn = [featp.tile([128, F], BF16, tag=f"vhn{mt}", name=f"vhn{mt}") for mt in range(NT)]
                for mt in range(NT):
                    pv = ps_tile([128, F])
                    for s in range(S):
                        nc.tensor.matmul(
                            pv[:, s * D:(s + 1) * D],
                            lhsT=vTq[mt][:, s * 128:(s + 1) * 128],
                            rhs=wv_sb[:, s * D:(s + 1) * D],
                            start=True, stop=True,
                        )
                    nc.vector.tensor_copy(out=vhn[mt][:, D:F], in_=pv[:, D:F])
                    nc.vector.tensor_tensor(
                        out=vhn[mt][:, 0:D],
                        in0=pv[:, 0:D], in1=bvrow_sb[:], op=ALU.add,
                    )

                # ---------------- exp + denominator accumulation, staggered
                exp_sf = [featp.tile([128, N], F32, tag=f"esf{mt}", name=f"esf{mt}") for mt in range(NT)]
                pden = ps_tile([NSEG, N])
                nc.tensor.matmul(
                    pden[:], lhsT=eps16[:].bitcast(F32R),
                    rhs=ones_n[:].bitcast(F32R), start=True, stop=False,
                    skip_group_check=True,
                )
                for mt in range(NT):
                    nc.scalar.activation(exp_sf[mt][:].bitcast(F32R), psf[mt][:],
                                         AF.Exp, scale=SCALE)
                    nc.tensor.matmul(
                        pden[:], lhsT=d_t[:, mt * NSEG:(mt + 1) * NSEG].bitcast(F32R),
                        rhs=exp_sf[mt][:].bitcast(F32R),
                        start=False, stop=(mt == NT - 1),
                        skip_group_check=True,
                    )
                dd = featp.tile([NSEG, N], F32, tag="dd", name="dd")
                with nc.allow_low_precision(reason="f32r is 32-bit storage"):
                    nc.vector.reciprocal(dd[:].bitcast(F32R), pden[:])

                # ------- per m-tile: aggt = exp_sf * (C^T dd); vhn; att-out
                # po[nt] accumulates mt-major in the acc banks (freed by exp)
                aggt = [featp.tile([128, N], BF16, tag=f"aggt{mt}", name=f"aggt{mt}") for mt in range(NT)]
                po = [pop.tile([128, F], F32, tag=f"acc{nt}", name=f"po{nt}")
                      for nt in range(NT)]
                agg_engines = [nc.vector, nc.vector]
                for mt in range(NT):
                    pT = ps_tile([128, N])
                    nc.tensor.matmul(
                        pT[:], lhsT=c_sb[:, mt * 128:(mt + 1) * 128].bitcast(F32R),
                        rhs=dd[:].bitcast(F32R),
                        start=True, stop=True,
                    )
                    agg_engines[mt % 2].tensor_tensor(
                        out=aggt[mt][:], in0=exp_sf[mt][:],
                        in1=pT[:], op=ALU.mult)
                    for nt in range(NT):
                        nc.tensor.matmul(
                            po[nt][:],
                            lhsT=aggt[mt][:, nt * 128:(nt + 1) * 128],
                            rhs=vhn[mt][:],
                            start=(mt == 0), stop=(mt == NT - 1),
                            skip_group_check=True,
                        )
                ao = aop.tile([128, NT * F], BF16, tag="ao", name="ao")
                for nt in range(NT):
                    copy_alt(ao[:, nt * F:(nt + 1) * F], po[nt][:])
                nc.sync.dma_start(
                    ao_d[:].rearrange("(t p) f -> p t f", t=NT),
                    ao[:].rearrange("p (t f) -> p t f", t=NT))

    _split_multiwaits(nc)
    return nc


def build_phase2(loop_R: int | None = None) -> bass.Bass:
    """Equivariant layernorm + output projection on a 64-atom slice.
    Input lnin [64, (s, ch)]; output yT [ci, (s, n)] (host un-transposes).
    gamma/beta are folded into the post-transpose PSUM->SBUF copies as
    per-partition tensor_scalar ops; same NEFF on all cores."""
    nc = bass.Bass("TRN2", target_bir_lowering=False, debug=False, num_devices=H)
    lnin_d = nc.dram_tensor("lnin", [NR, S * CH], BF16, kind="ExternalInput")
    gcol_d = nc.dram_tensor("gcol", [128, 2 * (LMAX + 1)], F32, kind="ExternalInput")
    bcol_d = nc.dram_tensor("bcol", [128, 2], BF16, kind="ExternalInput")
    # compact per-l output weights: [c_half, i, (l, ci)]
    woe_d = nc.dram_tensor("woe", [2, 128, (LMAX + 1) * CIN], BF16, kind="ExternalInput")
    bo_d = nc.dram_tensor("bo", [CIN, 1], F32, kind="ExternalInput")
    y_d = nc.dram_tensor("yT", [CIN, S * NR], F32, kind="ExternalOutput")

    with tile.TileContext(nc) as tc:
        with (
            tc.tile_pool(name="const", bufs=1) as cpool,
            tc.tile_pool(name="work", bufs=1) as workp,
            tc.tile_pool(name="tp", bufs=4) as tpp,
            tc.tile_pool(name="ps", bufs=1, space="PSUM") as psp,
            tc.tile_pool(name="plbp", bufs=4, space="PSUM") as plbp,
            tc.tile_pool(name="pyg", bufs=3, space="PSUM") as pygp,
        ):
            def ps_tile(shape):
                return psp.tile(shape, F32, tag="ps", name="ps")

            lnin = workp.tile([NR, S * CH], BF16, tag="lnin", name="lnin")
            # section DMAs so the l=0 chain starts early
            nc.sync.dma_start(lnin[:, 0:CH], lnin_d[:, 0:CH])
            nc.sync.dma_start(lnin[:, 4 * CH:S * CH], lnin_d[:, 4 * CH:S * CH])
            nc.sync.dma_start(lnin[:, CH:4 * CH], lnin_d[:, CH:4 * CH])
            woe_sb = [
                cpool.tile([128, (LMAX + 1) * CIN], BF16, tag=f"woe{c}", name=f"woe{c}")
                for c in range(2)
            ]
            gcol_sb = cpool.tile([128, 2 * (LMAX + 1)], F32, tag="gcol", name="gcol")
            bcol_sb = cpool.tile([128, 2], BF16, tag="bcol", name="bcol")
            bo_sb = cpool.tile([CIN, 1], F32, tag="bo", name="bo")
            for c in range(2):
                nc.sync.dma_start(woe_sb[c][:], woe_d[c, :, :])
            nc.sync.dma_start(gcol_sb[:], gcol_d[:])
            nc.sync.dma_start(bcol_sb[:], bcol_d[:])
            nc.sync.dma_start(bo_sb[:], bo_d[:])
            ident = cpool.tile([128, 128], BF16, tag="ident", name="ident")
            make_identity(nc, ident[:])
            eps_sb = cpool.tile([128, 1], F32, tag="epsc", name="epsc")
            nc.gpsimd.memset(eps_sb[:], EPS)

            ones_r = cpool.tile([NR, 128], BF16, tag="ones_r", name="ones_r")
            nc.gpsimd.memset(ones_r[:], 1.0)

            import contextlib as _ctl
            _loop = tc.For_i(0, loop_R, 1) if loop_R else _ctl.nullcontext()
            with _loop:
                # ---- LN statistics (per-atom scalars)
                # l=0: one-pass LN over CH: var = E[x^2] - mu^2
                x0 = lnin[:, 0:CH]
                sc0 = workp.tile([NR, CH], F32, tag="sc0", name="sc0")
                mu = workp.tile([NR, 1], F32, tag="mu", name="mu")
                nc.scalar.activation(sc0[:], x0, AF.Copy, scale=1.0 / CH,
                                     accum_out=mu[:])
                sq0 = workp.tile([NR, CH], F32, tag="sq0", name="sq0")
                vs = workp.tile([NR, 1], F32, tag="vs", name="vs")
                nc.vector.tensor_tensor(out=sq0[:], in0=x0, in1=x0, op=ALU.mult)
                nc.vector.reduce_sum(out=vs[:], in_=sq0[:],
                                     axis=mybir.AxisListType.X)
                mu2 = workp.tile([NR, 1], F32, tag="mu2", name="mu2")
                nc.gpsimd.tensor_tensor(out=mu2[:], in0=mu[:], in1=mu[:], op=ALU.mult)
                ebias = workp.tile([NR, 1], F32, tag="ebias", name="ebias")
                nc.gpsimd.tensor_scalar(out=ebias[:], in0=mu2[:], scalar1=-1.0,
                                        scalar2=EPS, op0=ALU.mult, op1=ALU.add)
                sd = workp.tile([NR, 1], F32, tag="sd", name="sd")
                nc.scalar.activation(sd[:], vs[:], AF.Sqrt, scale=1.0 / CH,
                                     bias=ebias[:, 0:1])
                rstd = workp.tile([NR, 1], F32, tag="rstd", name="rstd")
                nc.vector.reciprocal(rstd[:], sd[:])
                mr = workp.tile([NR, 1], F32, tag="mr", name="mr")
                nc.gpsimd.tensor_tensor(out=mr[:], in0=mu[:], in1=rstd[:], op=ALU.mult)
                nmr = workp.tile([NR, 1], F32, tag="nmr", name="nmr")
                nc.gpsimd.tensor_scalar_mul(nmr[:], mr[:], -1.0)
                # l=2 then l=1 RMS stats (Act Square+accum; recips on DVE)
                rr = {}
                for l in (2, 1):
                    lo, hi = (l * l) * CH, ((l + 1) * (l + 1)) * CH
                    width = hi - lo
                    ms = workp.tile([NR, 1], F32, tag=f"ms{l}", name=f"ms{l}")
                    sql = workp.tile([NR, width], F32, tag=f"sq{l}", name=f"sq{l}")
                    nc.scalar.activation(sql[:], lnin[:, lo:hi], AF.Square,
                                         accum_out=ms[:])
                    sdl = workp.tile([NR, 1], F32, tag=f"sd{l}", name=f"sd{l}")
                    nc.scalar.activation(sdl[:], ms[:], AF.Sqrt, scale=1.0 / width,
                                         bias=eps_sb[0:NR, 0:1])
                    rrl = workp.tile([NR, 1], F32, tag=f"rr{l}", name=f"rr{l}")
                    nc.vector.reciprocal(rrl[:], sdl[:])
                    rr[l] = rrl

                # ---- gamma folded into weights (DVE bf16 fast mode)
                woe_g = [workp.tile([128, (LMAX + 1) * CIN], BF16, tag=f"wg{c}",
                                    name=f"wg{c}") for c in range(2)]
                for c in range(2):
                    for l in range(LMAX + 1):
                        nc.vector.tensor_scalar_mul(
                            woe_g[c][:, l * CIN:(l + 1) * CIN],
                            woe_sb[c][:, l * CIN:(l + 1) * CIN],
                            gcol_sb[:, 2 * l + c:2 * l + c + 1])
                # beta contribution: pbw[ci] = sum_ch beta[ch] * Wo[0][ch, ci]
                pbw = ps_tile([CIN, 1])
                for c in range(2):
                    nc.tensor.matmul(
                        pbw[:], lhsT=woe_sb[c][:, 0:CIN],
                        rhs=bcol_sb[:, c:c + 1],
                        start=(c == 0), stop=(c == 1))
                bo0 = workp.tile([CIN, 1], F32, tag="bo0", name="bo0")
                nc.vector.tensor_tensor(out=bo0[:], in0=pbw[:], in1=bo_sb[:],
                                        op=ALU.add)

                # ---- diag(scale) tiles: the transpose matmul applies the
                # per-atom LN scaling for free (rhs = diag instead of I)
                diag = {}
                for l, scl in ((2, rr[2]), (0, rstd), (1, rr[1])):
                    dg = workp.tile([NR, NR], BF16, tag=f"diag{l}", name=f"diag{l}")
                    nc.gpsimd.tensor_scalar_mul(dg[:], ident[0:NR, 0:NR],
                                                scl[:, 0:1])
                    diag[l] = dg
                dnm = workp.tile([NR, NR], BF16, tag="dnm", name="dnm")
                nc.gpsimd.tensor_scalar_mul(dnm[:], ident[0:NR, 0:NR], nmr[:, 0:1])

                # ---- per s (l=2 block first): scale+transpose -> lnT -> yT
                y_sb = workp.tile([CIN, S * NR], F32, tag="ysb", name="ysb")
                out_dma = [nc.sync, nc.scalar]
                for si, s in enumerate([4, 5, 6, 7, 8, 0, 1, 2, 3]):
                    l = int(L_OF_M[s])
                    pl = plbp.tile([128, 2 * NR], F32, tag="plb", name="plb")
                    for c in range(2):
                        nc.tensor.matmul(
                            pl[:, c * NR:(c + 1) * NR],
                            lhsT=lnin[:, s * CH + c * 128: s * CH + (c + 1) * 128],
                            rhs=diag[l][:],
                            start=True, stop=(s > 0),
                            skip_group_check=True,
                        )
                        if s == 0:
                            nc.tensor.matmul(
                                pl[:, c * NR:(c + 1) * NR],
                                lhsT=ones_r[:],
                                rhs=dnm[:],
                                start=False, stop=True,
                                skip_group_check=True,
                            )
                    lnT = tpp.tile([128, 2 * NR], BF16, tag="lnT", name="lnT")
                    if si % 2:
                        nc.vector.tensor_copy(out=lnT[:], in_=pl[:])
                    else:
                        nc.scalar.copy(lnT[:], pl[:])
                    py = pygp.tile([CIN, NR], F32, tag="pyg", name="pyg")
                    for c in range(2):
                        nc.tensor.matmul(
                            py[:],
                            lhsT=woe_g[c][:, l * CIN:(l + 1) * CIN],
                            rhs=lnT[:, c * NR:(c + 1) * NR],
                            start=(c == 0), stop=(c == 1),
                        )
                    bias = bo0[:, 0:1] if s == 0 else bo_sb[:, 0:1]
                    if si % 2:
                        nc.vector.tensor_scalar_add(y_sb[:, s * NR:(s + 1) * NR],
                                                    py[:], bias)
                    else:
                        nc.scalar.activation(y_sb[:, s * NR:(s + 1) * NR], py[:],
                                             AF.Identity, bias=bias)
                    if s == 8:
                        out_dma[0].dma_start(y_d[:, 4 * NR:S * NR],
                                             y_sb[:, 4 * NR:S * NR])
                    elif s == 3:
                        out_dma[1].dma_start(y_d[:, 0:4 * NR],
                                             y_sb[:, 0:4 * NR])

    _split_multiwaits(nc)
    return nc


# ------------------------------------------------------------------ host side
def _prep_inputs(inputs: dict[str, np.ndarray]):
    """Split the full inputs into per-core in_maps for the two phases
    (index bookkeeping and value re-layout only; all arithmetic on device)."""
    q = np.asarray(inputs["q"], np.float32).reshape(N, S, CIN)
    k = np.asarray(inputs["k"], np.float32).reshape(N, S, CIN)
    v = np.asarray(inputs["v"], np.float32).reshape(N, S, CIN)
    # host pre-transpose to [i, (s, m)] and cast to bf16; vT mt-major
    qT = np.ascontiguousarray(q.transpose(2, 1, 0).reshape(CIN, S * N)).astype(NP_BF16)
    kT = np.ascontiguousarray(k.transpose(2, 1, 0).reshape(CIN, S * N)).astype(NP_BF16)
    vT = np.ascontiguousarray(
        v.reshape(NT, 128, S, CIN).transpose(3, 0, 2, 1).reshape(CIN, S * N)
    ).astype(NP_BF16)
    env = np.asarray(inputs["envelope"], np.float32)
    attn_bias = np.asarray(inputs["attn_bias"], np.float32)
    a_idx = np.asarray(inputs["atom_index"]).astype(np.int64)
    b_idx = np.asarray(inputs["batch_index"]).astype(np.int64)
    e_map = np.asarray(inputs["edge_map_tab"]).astype(np.int64)
    Wq = np.asarray(inputs["Wq"], np.float32)
    Wk = np.asarray(inputs["Wk"], np.float32)
    Wv = np.asarray(inputs["Wv"], np.float32)
    bq = np.asarray(inputs["bq"], np.float32)
    bk = np.asarray(inputs["bk"], np.float32)
    bv = np.asarray(inputs["bv"], np.float32)
    gamma = np.asarray(inputs["gamma"], np.float32)
    beta = np.asarray(inputs["beta"], np.float32)
    Wo = np.asarray(inputs["Wo"], np.float32)
    bo = np.asarray(inputs["bo"], np.float32)

    # ---- slot layout for the (atom, segment) cells
    cell = a_idx * NSEG + b_idx                      # [E]
    order = np.argsort(cell, kind="stable")
    cell_s = cell[order]
    counts = np.bincount(cell_s, minlength=N * NSEG)
    L2 = int(counts.max())
    starts = np.zeros(N * NSEG, np.int64)
    starts[1:] = np.cumsum(counts)[:-1]
    rank = np.arange(E) - starts[cell_s]             # rank within cell
    m_s = cell_s // NSEG
    g_s = cell_s % NSEG
    p_s = m_s % 128
    t_s = m_s // 128
    col = (t_s * NSEG + g_s) * L2 + rank             # free-dim position
    Wd = NT * NSEG * L2
    env_e = env[e_map]                               # value gather (re-layout)
    envS = np.zeros((128, Wd), np.float32)
    envS[p_s, col] = env_e[order]
    bS_all = []
    for h in range(H):
        bs = np.zeros((128, Wd), np.float32)
        bs[p_s, col] = attn_bias[h, e_map][order]
        bS_all.append(bs)

    # ---- per-head weight slices, expanded per spherical component, [i,(s,o)]
    WqE = Wq[L_OF_M]                                 # [9, CIN, CH]
    WkE = Wk[L_OF_M]
    WvE = Wv[L_OF_M]

    in_maps1 = []
    for h in range(H):
        sl = slice(h * D, (h + 1) * D)
        in_maps1.append({
            "qT": qT, "kT": kT, "vT": vT,
            "wq": np.ascontiguousarray(
                WqE[:, :, sl].transpose(1, 0, 2).reshape(CIN, S * D)).astype(NP_BF16),
            "wk": np.ascontiguousarray(
                WkE[:, :, sl].transpose(1, 0, 2).reshape(CIN, S * D)).astype(NP_BF16),
            "wv": np.ascontiguousarray(
                WvE[:, :, sl].transpose(1, 0, 2).reshape(CIN, S * D)).astype(NP_BF16),
            "bqkv": np.ascontiguousarray(
                np.stack([bq[sl], bk[sl], bv[sl]], axis=1)
            ),
            "bvrow": np.ascontiguousarray(bv[sl].reshape(1, D)),
            "envs": envS,
            "bs": bS_all[h],
        })

    # ---- phase-2 constants
    # gcol[p, 2l+c] = gamma[l, c*128+p];  bcol[p, c] = beta[c*128+p]
    gcol = np.zeros((128, 2 * (LMAX + 1)), np.float32)
    for l in range(LMAX + 1):
        for c in range(2):
            gcol[:, 2 * l + c] = gamma[l, c * 128:(c + 1) * 128]
    bcol = np.stack([beta[0:128], beta[128:256]], axis=1).astype(NP_BF16)
    woe = np.zeros((2, 128, (LMAX + 1) * CIN), NP_BF16)
    for c in range(2):
        woe[c] = Wo[:, c * 128:(c + 1) * 128, :].transpose(1, 0, 2).reshape(
            128, (LMAX + 1) * CIN).astype(NP_BF16)
    p2_const = {"gcol": gcol, "bcol": bcol, "woe": woe,
                "bo": np.ascontiguousarray(bo.reshape(CIN, 1))}
    return in_maps1, L2, p2_const


def _reorder_ao(ao_all: list[np.ndarray]) -> list[np.ndarray]:
    """[h][N, (s,d)] -> per-core [64, (s, h*D+d)] slices (pure data movement)."""
    full = np.stack([np.asarray(a).reshape(N, S, D) for a in ao_all], axis=2)
    full = full.reshape(N, S * CH)                                # [N, (S, H*D)]
    return [np.ascontiguousarray(full[c * NR:(c + 1) * NR]).astype(NP_BF16)
            for c in range(H)]


_BUILD_CACHE: dict = {}


def kernel(**inputs) -> np.ndarray:
    in_maps1, L2, p2_const = _prep_inputs(inputs)
    nc1 = _BUILD_CACHE.get(("p1", L2))
    if nc1 is None:
        nc1 = build_bass(L2)
        _BUILD_CACHE[("p1", L2)] = nc1
    res1 = run_bass_kernel_spmd(nc1, in_maps1, core_ids=list(range(H)))
    lnin_slices = _reorder_ao([r["ao"] for r in res1.results])

    nc2 = _BUILD_CACHE.get("p2")
    if nc2 is None:
        nc2 = build_phase2()
        _BUILD_CACHE["p2"] = nc2
    in_maps2 = [{"lnin": lnin_slices[c], **p2_const} for c in range(H)]
    res2 = run_bass_kernel_spmd(nc2, in_maps2, core_ids=list(range(H)))
    # yT [ci, (s, n_local)] per core -> y [N, S, CIN]
    y = np.zeros((N, S, CIN), np.float32)
    for c in range(H):
        yt = res2.results[c]["yT"].reshape(CIN, S, NR)
        y[c * NR:(c + 1) * NR] = yt.transpose(2, 1, 0)
    return np.ascontiguousarray(y)


# revision 71
# speedup vs baseline: 2.1808x; 1.0018x over previous
"""Equivariant attention (gnn_message_passing) on 8 Trainium2 NeuronCores.

Strategy (head-sharded tensor parallel, core c owns head c):

The reference materializes [H, N, E] scores/attn over E=8192 edges. Here the
edge dimension is collapsed onto the N=512 atoms at projection level:

  scores[h, n, e]   = sf[h, n, a_e] + bias[h, edge_map[e]]     (a_e = atom_index)
  attn-softmax per (batch-segment, n) then  out = attn @ vh_edges

factors exactly into dense [N, N] algebra with two tiny per-(segment, atom)
tables (NSEG=16 x N=512):

  D[g, m] = sum_{e in seg g, a_e = m} env_e   * exp(b_e)
  C[g, m] = sum_{e in seg g, a_e = m} env_e^2 * exp(b_e)
  den[g, n]  = sum_m exp(sf[m, n]) * D[g, m]           (one matmul)
  Aagg[m, n] = exp(sf[m, n]) * sum_g C[g, m] / den[g, n]
  out[n, f]  = Aagg^T @ vh[m, f]                        (one matmul)

The running-max subtraction in the reference softmax cancels exactly (up to a
+1e-16 epsilon whose relative effect is ~1e-16) and |scale*sf + b| < 20, so
unnormalized exp is safe in f32.

D/C are built on-device from "slot tensors": host packs per-(atom, segment)
edge lists into a fixed-width [128, 4*16*L2] layout (env & bias values; pads
have env=0 so they vanish), and a single free-axis reduce per table produces
it. Only integer index bookkeeping and value re-layout happen on host.

q/k/v arrive HOST-PRE-TRANSPOSED as qT/kT/vT [CIN, S*N] (channel-major), so
the kernel needs no on-device input transposes: projections read qT slices
directly.  DMAs are issued in first-use order so the PE starts ~1.5us in.

Phase 2 (per-core 64-atom slice): LN + output projection, with the output
projection done in yT [ci, (s, n)] orientation (gamma/beta folded into the
post-transpose PSUM copy); host un-transposes for free.
"""

import os
import numpy as np

import concourse.bass as bass
import concourse.tile as tile
from concourse import mybir
from concourse.bass_utils import run_bass_kernel_spmd
from concourse.masks import make_identity

# ---------------------------------------------------------------- constants
H, LMAX, NSEG = 8, 2, 16
S = (LMAX + 1) ** 2          # 9 spherical components
N, E, CIN, CH = 512, 8192, 128, 256
D = CH // H                  # 32 per-head channels
F = S * D                    # 288 per-head feature width
NT = N // 128                # 4 atom tiles
NR = N // H                  # 64 atoms per core in the LN/out stage
EPS = 1e-7
SCALE = float(np.sqrt(D / 3.0) / D)
L_OF_M = np.floor(np.sqrt(np.arange(S))).astype(np.int64)
F32 = mybir.dt.float32
F32R = mybir.dt.float32r
BF16 = mybir.dt.bfloat16
AF = mybir.ActivationFunctionType
ALU = mybir.AluOpType

import ml_dtypes
NP_BF16 = ml_dtypes.bfloat16

_DBG = bool(int(os.environ.get("KBDBG", "0")))


def _split_multiwaits(nc: bass.Bass, limit: int = 1):
    """This walrus build rejects instructions carrying more than one semaphore
    wait (and Drains carrying any). Hoist excess waits onto NOPs inserted just
    before the instruction on the same engine - semantically identical."""
    for f in nc.m.functions:
        for blk in f.blocks:
            changed = False
            out = []
            for inst in blk.instructions:
                si = inst.sync_info
                waits = list(si.on_wait) if si is not None else []
                keep = 0 if inst.opcode == "Drain" else limit
                if len(waits) > keep:
                    hoist = waits[: len(waits) - keep]
                    rest = waits[len(waits) - keep:]
                    for w in hoist:
                        nop = mybir.InstNoOp(
                            name=f"{inst.name}-w{len(out)}", ins=[], outs=[]
                        )
                        nop.engine = inst.engine
                        nop.sync_info = mybir.SyncInfo(on_wait=[w], on_update=[])
                        out.append(nop)
                    inst.sync_info = mybir.SyncInfo(
                        on_wait=rest, on_update=list(si.on_update)
                    )
                    changed = True
                out.append(inst)
            if changed:
                blk.instructions = out


def build_bass(L2: int, loop_R: int | None = None) -> bass.Bass:
    """One SPMD program; per-core data (weight slices, bias slots) comes in as
    inputs. L2 = slot width per (atom, segment) cell."""
    W = NT * NSEG * L2  # slot tensor free width per partition

    nc = bass.Bass("TRN2", target_bir_lowering=False, debug=False, num_devices=H)

    # ------------------------------------------------------------- tensors
    # host-pre-transposed bf16 inputs: qT/kT [i, (s, m)]; vT mt-major
    # [i, (t, s, j)] so each quarter is a contiguous DMA
    qT_d = nc.dram_tensor("qT", [CIN, S * N], BF16, kind="ExternalInput")
    kT_d = nc.dram_tensor("kT", [CIN, S * N], BF16, kind="ExternalInput")
    vT_d = nc.dram_tensor("vT", [CIN, S * N], BF16, kind="ExternalInput")
    wq_d = nc.dram_tensor("wq", [CIN, S * D], BF16, kind="ExternalInput")  # [i,(s,o)]
    wk_d = nc.dram_tensor("wk", [CIN, S * D], BF16, kind="ExternalInput")
    wv_d = nc.dram_tensor("wv", [CIN, S * D], BF16, kind="ExternalInput")
    bqkv_d = nc.dram_tensor("bqkv", [D, 3], F32, kind="ExternalInput")
    bvrow_d = nc.dram_tensor("bvrow", [1, D], F32, kind="ExternalInput")
    envs_d = nc.dram_tensor("envs", [128, W], F32, kind="ExternalInput")
    bs_d = nc.dram_tensor("bs", [128, W], F32, kind="ExternalInput")
    ao_d = nc.dram_tensor("ao", [N, F], BF16, kind="ExternalOutput")

    with tile.TileContext(nc) as tc:
        with (
            tc.tile_pool(name="const", bufs=1) as cpool,
            tc.tile_pool(name="feat", bufs=1) as featp,
            tc.tile_pool(name="work", bufs=1) as workp,
            tc.tile_pool(name="aop", bufs=1) as aop,
            tc.tile_pool(name="ps", bufs=4, space="PSUM") as psp,
            tc.tile_pool(name="pop", bufs=1, space="PSUM") as pop,
        ):
            def ps_tile(shape):
                return psp.tile(shape, F32, tag="ps", name="ps")

            # -------- DMA issue, first-use order, spread over engines ------
            # sync queue:   wq, qT thirds, vT quarters (mt-major layout)
            # scalar queue: wk, wv, kT thirds, envs, bs
            # gpsimd queue: tiny constants
            TH = 3 * N
            wq_sb = cpool.tile([CIN, S * D], BF16, tag="wq", name="wq")
            wk_sb = cpool.tile([CIN, S * D], BF16, tag="wk", name="wk")
            wv_sb = cpool.tile([CIN, S * D], BF16, tag="wv", name="wv")
            qTc = [cpool.tile([CIN, TH], BF16, tag=f"qT{t}", name=f"qT{t}")
                   for t in range(3)]
            kTc = [cpool.tile([CIN, TH], BF16, tag=f"kT{t}", name=f"kT{t}")
                   for t in range(3)]
            # vT quarter mt holds [i, (s, m_local)] for atom tile mt
            vTq = [cpool.tile([CIN, S * 128], BF16, tag=f"vT{m}", name=f"vT{m}")
                   for m in range(NT)]
            envs_sb = cpool.tile([128, W], F32, tag="envs", name="envs")
            bs_sb = cpool.tile([128, W], F32, tag="bs", name="bs")

            def vtq_src(m):
                return vT_d[:].rearrange("i (t x) -> i t x", t=NT)[:, m, :]

            nc.sync.dma_start(wq_sb[:], wq_d[:])
            for t in range(3):
                nc.sync.dma_start(qTc[t][:], qT_d[:, t * TH:(t + 1) * TH])
            nc.sync.dma_start(vTq[0][:], vtq_src(0))
            nc.sync.dma_start(vTq[1][:], vtq_src(1))
            nc.scalar.dma_start(wk_sb[:], wk_d[:])
            for t in range(3):
                nc.scalar.dma_start(kTc[t][:], kT_d[:, t * TH:(t + 1) * TH])
            nc.scalar.dma_start(wv_sb[:], wv_d[:])
            nc.gpsimd.dma_start(envs_sb[:], envs_d[:])
            nc.gpsimd.dma_start(bs_sb[:], bs_d[:])
            nc.scalar.dma_start(vTq[2][:], vtq_src(2))
            nc.scalar.dma_start(vTq[3][:], vtq_src(3))
            bqkv_sb = cpool.tile([D, 3], F32, tag="bqkv", name="bqkv")
            nc.gpsimd.dma_start(bqkv_sb[:], bqkv_d[:])
            bvrow_sb = cpool.tile([128, D], F32, tag="bvrow", name="bvrow")
            nc.gpsimd.dma_start(bvrow_sb[:], bvrow_d[0:1, :].to_broadcast([128, D]))
            ident = cpool.tile([128, 128], F32, tag="ident", name="ident")
            make_identity(nc, ident[:])
            eps16 = cpool.tile([1, NSEG], F32, tag="eps16", name="eps16")
            nc.gpsimd.memset(eps16[:], 1e-16)
            ones_n = cpool.tile([1, N], F32, tag="ones_n", name="ones_n")
            nc.gpsimd.memset(ones_n[:], 1.0)

            import contextlib as _ctl
            _loop = tc.For_i(0, loop_R, 1) if loop_R else _ctl.nullcontext()
            with _loop:
                copy_engines = [nc.scalar, nc.vector]
                cp_i = 0

                def copy_alt(dst_ap, src_ap):
                    nonlocal cp_i
                    eng = copy_engines[cp_i % 2]
                    cp_i += 1
                    if eng is nc.scalar:
                        eng.copy(dst_ap, src_ap)
                    else:
                        eng.tensor_copy(out=dst_ap, in_=src_ap)

                def copy_dve(dst_ap, src_ap):
                    nc.vector.tensor_copy(out=dst_ap, in_=src_ap)

                # ------------------------------ D / C tables (early, off-path)
                ebs = workp.tile([128, W], F32, tag="ebs", name="ebs")
                nc.scalar.activation(ebs[:], bs_sb[:], AF.Exp)
                wD = workp.tile([128, W], F32, tag="wD", name="wD")
                nc.vector.tensor_tensor(out=wD[:], in0=envs_sb[:], in1=ebs[:], op=ALU.mult)
                wC = workp.tile([128, W], F32, tag="wC", name="wC")
                nc.vector.tensor_tensor(out=wC[:], in0=wD[:], in1=envs_sb[:], op=ALU.mult)
                d_t = featp.tile([128, NT * NSEG], F32, tag="d_t", name="d_t")  # [m_p, (mt, g)]
                c_t = featp.tile([128, NT * NSEG], F32, tag="c_t", name="c_t")
                with nc.allow_low_precision(reason="f32r is 32-bit storage"):
                    nc.vector.reduce_sum(
                        out=d_t[:].rearrange("p (t g) -> p t g", t=NT).bitcast(F32R),
                        in_=wD[:].rearrange("p (t g j) -> p t g j", t=NT, g=NSEG),
                        axis=mybir.AxisListType.X,
                    )
                nc.vector.reduce_sum(
                    out=c_t[:].rearrange("p (t g) -> p t g", t=NT),
                    in_=wC[:].rearrange("p (t g j) -> p t g j", t=NT, g=NSEG),
                    axis=mybir.AxisListType.X,
                )
                # ------- fq / fk chunk-streamed projection + psf accumulation
                # chunk layout: rows (s_local*32+o), chunks s=0..2 / 3..5 / 6..8
                # (96 rows per chunk so matmul outs land at base 0/32/64);
                # psf[mt] accumulates across chunks in 4 held banks (tags
                # shared with the po accumulators, which start strictly later)
                fq = [featp.tile([96, N], BF16, tag=f"fq{c}", name=f"fq{c}")
                      for c in range(3)]
                fk = [featp.tile([96, N], BF16, tag=f"fk{c}", name=f"fk{c}")
                      for c in range(3)]
                psf = [pop.tile([128, N], F32, tag=f"acc{mt}", name=f"psf{mt}")
                       for mt in range(NT)]
                # within chunk 0 the s components sit in row order (1, 2, 0)
                # so the biased s=0 rows are 64-aligned for the PSUM read
                # (fk uses the same permutation, so scores are unchanged)
                ROWOF = {0: 2, 1: 0, 2: 1}
                for chunk in range(3):
                    for t_c, w_sb, f_dst, t_idx in ((qTc, wq_sb, fq, 0),
                                                    (kTc, wk_sb, fk, 1)):
                        pp = ps_tile([96, N])
                        for j in range(3):
                            s = chunk * 3 + j
                            r = ROWOF[j] if chunk == 0 else j
                            nc.tensor.matmul(
                                pp[r * D:(r + 1) * D, :],
                                lhsT=w_sb[:, s * D:(s + 1) * D],
                                rhs=t_c[chunk][:, j * N:(j + 1) * N],
                                start=True, stop=True,
                            )
                        if chunk == 0:
                            # bias on s=0 rows (l=0 invariant component)
                            copy_dve(f_dst[0][0:64, :], pp[0:64, :])
                            nc.vector.tensor_scalar_add(
                                f_dst[0][64:96, :], pp[64:96, :],
                                bqkv_sb[:, t_idx:t_idx + 1])
                        else:
                            copy_dve(f_dst[chunk][:], pp[:])
                    for mt in range(NT):
                        nc.tensor.matmul(
                            psf[mt][:],
                            lhsT=fk[chunk][:, mt * 128:(mt + 1) * 128],
                            rhs=fq[chunk][:],
                            start=(chunk == 0), stop=(chunk == 2),
                            skip_group_check=True,
                        )

                # C transposed to [g, m]
                c_sb = featp.tile([NSEG, N], F32, tag="c_sb", name="c_sb")
                for mt in range(NT):
                    pc = ps_tile([NSEG, 128])
                    nc.tensor.transpose(
                        pc[:], c_t[:, mt * NSEG:(mt + 1) * NSEG], ident[:]
                    )
                    nc.vector.tensor_copy(out=c_sb[:, mt * 128:(mt + 1) * 128].bitcast(F32R), in_=pc[:])

                # -------------------------------- vhn [m, (s,o)] per m-tile
                vhn = [featp.tile([128, F], BF16, tag=f"vhn{mt}", name=f"vhn{mt}") for mt in range(NT)]
                for mt in range(NT):
                    pv = ps_tile([128, F])
                    for s in range(S):
                        nc.tensor.matmul(
                            pv[:, s * D:(s + 1) * D],
                            lhsT=vTq[mt][:, s * 128:(s + 1) * 128],
                            rhs=wv_sb[:, s * D:(s + 1) * D],
                            start=True, stop=True,
                        )
                    nc.vector.tensor_copy(out=vhn[mt][:, D:F], in_=pv[:, D:F])
                    nc.vector.tensor_tensor(
                        out=vhn[mt][:, 0:D],
                        in0=pv[:, 0:D], in1=bvrow_sb[:], op=ALU.add,
                    )

                # ---------------- exp + denominator accumulation, staggered
                exp_sf = [featp.tile([128, N], F32, tag=f"esf{mt}", name=f"esf{mt}") for mt in range(NT)]
                pden = ps_tile([NSEG, N])
                nc.tensor.matmul(
                    pden[:], lhsT=eps16[:].bitcast(F32R),
                    rhs=ones_n[:].bitcast(F32R), start=True, stop=False,
                    skip_group_check=True,
                )
                for mt in range(NT):
                    nc.scalar.activation(exp_sf[mt][:].bitcast(F32R), psf[mt][:],
                                         AF.Exp, scale=SCALE)
                    nc.tensor.matmul(
                        pden[:], lhsT=d_t[:, mt * NSEG:(mt + 1) * NSEG].bitcast(F32R),
                        rhs=exp_sf[mt][:].bitcast(F32R),
                        start=False, stop=(mt == NT - 1),
                        skip_group_check=True,
                    )
                dd = featp.tile([NSEG, N], F32, tag="dd", name="dd")
                with nc.allow_low_precision(reason="f32r is 32-bit storage"):
                    nc.vector.reciprocal(dd[:].bitcast(F32R), pden[:])

                # ------- per m-tile: aggt = exp_sf * (C^T dd); vhn; att-out
                # po[nt] accumulates mt-major in the acc banks (freed by exp)
                aggt = [featp.tile([128, N], BF16, tag=f"aggt{mt}", name=f"aggt{mt}") for mt in range(NT)]
                po = [pop.tile([128, F], F32, tag=f"acc{nt}", name=f"po{nt}")
                      for nt in range(NT)]
                agg_engines = [nc.vector, nc.vector]
                for mt in range(NT):
                    pT = ps_tile([128, N])
                    nc.tensor.matmul(
                        pT[:], lhsT=c_sb[:, mt * 128:(mt + 1) * 128].bitcast(F32R),
                        rhs=dd[:].bitcast(F32R),
                        start=True, stop=True,
                    )
                    agg_engines[mt % 2].tensor_tensor(
                        out=aggt[mt][:], in0=exp_sf[mt][:],
                        in1=pT[:], op=ALU.mult)
                    for nt in range(NT):
                        nc.tensor.matmul(
                            po[nt][:],
                            lhsT=aggt[mt][:, nt * 128:(nt + 1) * 128],
                            rhs=vhn[mt][:],
                            start=(mt == 0), stop=(mt == NT - 1),
                            skip_group_check=True,
                        )
                ao = aop.tile([128, NT * F], BF16, tag="ao", name="ao")
                for nt in range(NT):
                    copy_alt(ao[:, nt * F:(nt + 1) * F], po[nt][:])
                nc.sync.dma_start(
                    ao_d[:].rearrange("(t p) f -> p t f", t=NT),
                    ao[:].rearrange("p (t f) -> p t f", t=NT))

    _split_multiwaits(nc)
    return nc


def build_phase2(loop_R: int | None = None) -> bass.Bass:
    """Equivariant layernorm + output projection on a 64-atom slice.
    Input lnin [64, (s, ch)]; output yT [ci, (s, n)] (host un-transposes).
    gamma/beta are folded into the post-transpose PSUM->SBUF copies as
    per-partition tensor_scalar ops; same NEFF on all cores."""
    nc = bass.Bass("TRN2", target_bir_lowering=False, debug=False, num_devices=H)
    lnin_d = nc.dram_tensor("lnin", [NR, S * CH], BF16, kind="ExternalInput")
    gcol_d = nc.dram_tensor("gcol", [128, 2 * (LMAX + 1)], F32, kind="ExternalInput")
    bcol_d = nc.dram_tensor("bcol", [128, 2], BF16, kind="ExternalInput")
    # compact per-l output weights: [c_half, i, (l, ci)]
    woe_d = nc.dram_tensor("woe", [2, 128, (LMAX + 1) * CIN], BF16, kind="ExternalInput")
    bo_d = nc.dram_tensor("bo", [CIN, 1], F32, kind="ExternalInput")
    y_d = nc.dram_tensor("yT", [CIN, S * NR], F32, kind="ExternalOutput")

    with tile.TileContext(nc) as tc:
        with (
            tc.tile_pool(name="const", bufs=1) as cpool,
            tc.tile_pool(name="work", bufs=1) as workp,
            tc.tile_pool(name="tp", bufs=4) as tpp,
            tc.tile_pool(name="ps", bufs=1, space="PSUM") as psp,
            tc.tile_pool(name="plbp", bufs=4, space="PSUM") as plbp,
            tc.tile_pool(name="pyg", bufs=3, space="PSUM") as pygp,
        ):
            def ps_tile(shape):
                return psp.tile(shape, F32, tag="ps", name="ps")

            lnin = workp.tile([NR, S * CH], BF16, tag="lnin", name="lnin")
            # section DMAs so the l=0 chain starts early
            nc.sync.dma_start(lnin[:, 0:CH], lnin_d[:, 0:CH])
            nc.sync.dma_start(lnin[:, 4 * CH:S * CH], lnin_d[:, 4 * CH:S * CH])
            nc.sync.dma_start(lnin[:, CH:4 * CH], lnin_d[:, CH:4 * CH])
            woe_sb = [
                cpool.tile([128, (LMAX + 1) * CIN], BF16, tag=f"woe{c}", name=f"woe{c}")
                for c in range(2)
            ]
            gcol_sb = cpool.tile([128, 2 * (LMAX + 1)], F32, tag="gcol", name="gcol")
            bcol_sb = cpool.tile([128, 2], BF16, tag="bcol", name="bcol")
            bo_sb = cpool.tile([CIN, 1], F32, tag="bo", name="bo")
            for c in range(2):
                nc.sync.dma_start(woe_sb[c][:], woe_d[c, :, :])
            nc.sync.dma_start(gcol_sb[:], gcol_d[:])
            nc.sync.dma_start(bcol_sb[:], bcol_d[:])
            nc.sync.dma_start(bo_sb[:], bo_d[:])
            ident = cpool.tile([128, 128], BF16, tag="ident", name="ident")
            make_identity(nc, ident[:])
            eps_sb = cpool.tile([128, 1], F32, tag="epsc", name="epsc")
            nc.gpsimd.memset(eps_sb[:], EPS)

            ones_r = cpool.tile([NR, 128], BF16, tag="ones_r", name="ones_r")
            nc.gpsimd.memset(ones_r[:], 1.0)

            import contextlib as _ctl
            _loop = tc.For_i(0, loop_R, 1) if loop_R else _ctl.nullcontext()
            with _loop:
                # ---- LN statistics (per-atom scalars)
                # l=0: one-pass LN over CH: var = E[x^2] - mu^2
                x0 = lnin[:, 0:CH]
                sc0 = workp.tile([NR, CH], F32, tag="sc0", name="sc0")
                mu = workp.tile([NR, 1], F32, tag="mu", name="mu")
                nc.scalar.activation(sc0[:], x0, AF.Copy, scale=1.0 / CH,
                                     accum_out=mu[:])
                sq0 = workp.tile([NR, CH], F32, tag="sq0", name="sq0")
                vs = workp.tile([NR, 1], F32, tag="vs", name="vs")
                nc.vector.tensor_tensor(out=sq0[:], in0=x0, in1=x0, op=ALU.mult)
                nc.vector.reduce_sum(out=vs[:], in_=sq0[:],
                                     axis=mybir.AxisListType.X)
                mu2 = workp.tile([NR, 1], F32, tag="mu2", name="mu2")
                nc.gpsimd.tensor_tensor(out=mu2[:], in0=mu[:], in1=mu[:], op=ALU.mult)
                ebias = workp.tile([NR, 1], F32, tag="ebias", name="ebias")
                nc.gpsimd.tensor_scalar(out=ebias[:], in0=mu2[:], scalar1=-1.0,
                                        scalar2=EPS, op0=ALU.mult, op1=ALU.add)
                sd = workp.tile([NR, 1], F32, tag="sd", name="sd")
                nc.scalar.activation(sd[:], vs[:], AF.Sqrt, scale=1.0 / CH,
                                     bias=ebias[:, 0:1])
                rstd = workp.tile([NR, 1], F32, tag="rstd", name="rstd")
                nc.vector.reciprocal(rstd[:], sd[:])
                mr = workp.tile([NR, 1], F32, tag="mr", name="mr")
                nc.gpsimd.tensor_tensor(out=mr[:], in0=mu[:], in1=rstd[:], op=ALU.mult)
                nmr = workp.tile([NR, 1], F32, tag="nmr", name="nmr")
                nc.gpsimd.tensor_scalar_mul(nmr[:], mr[:], -1.0)
                # l=2 then l=1 RMS stats (Act Square+accum; recips on DVE)
                rr = {}
                for l in (2, 1):
                    lo, hi = (l * l) * CH, ((l + 1) * (l + 1)) * CH
                    width = hi - lo
                    ms = workp.tile([NR, 1], F32, tag=f"ms{l}", name=f"ms{l}")
                    if l == 2:
                        sql = workp.tile([NR, width], BF16, tag=f"sq{l}",
                                         name=f"sq{l}")
                        nc.vector.tensor_tensor(out=sql[:], in0=lnin[:, lo:hi],
                                                in1=lnin[:, lo:hi], op=ALU.mult)
                        with nc.allow_low_precision(reason="rms over 1280 terms"):
                            nc.vector.reduce_sum(out=ms[:], in_=sql[:],
                                                 axis=mybir.AxisListType.X)
                    else:
                        sql = workp.tile([NR, width], F32, tag=f"sq{l}",
                                         name=f"sq{l}")
                        nc.scalar.activation(sql[:], lnin[:, lo:hi], AF.Square,
                                             accum_out=ms[:])
                    sdl = workp.tile([NR, 1], F32, tag=f"sd{l}", name=f"sd{l}")
                    nc.scalar.activation(sdl[:], ms[:], AF.Sqrt, scale=1.0 / width,
                                         bias=eps_sb[0:NR, 0:1])
                    rrl = workp.tile([NR, 1], F32, tag=f"rr{l}", name=f"rr{l}")
                    nc.vector.reciprocal(rrl[:], sdl[:])
                    rr[l] = rrl

                # ---- gamma folded into weights (DVE bf16 fast mode)
                woe_g = [workp.tile([128, (LMAX + 1) * CIN], BF16, tag=f"wg{c}",
                                    name=f"wg{c}") for c in range(2)]
                for c in range(2):
                    for l in range(LMAX + 1):
                        nc.vector.tensor_scalar_mul(
                            woe_g[c][:, l * CIN:(l + 1) * CIN],
                            woe_sb[c][:, l * CIN:(l + 1) * CIN],
                            gcol_sb[:, 2 * l + c:2 * l + c + 1])
                # beta contribution: pbw[ci] = sum_ch beta[ch] * Wo[0][ch, ci]
                pbw = ps_tile([CIN, 1])
                for c in range(2):
                    nc.tensor.matmul(
                        pbw[:], lhsT=woe_sb[c][:, 0:CIN],
                        rhs=bcol_sb[:, c:c + 1],
                        start=(c == 0), stop=(c == 1))
                bo0 = workp.tile([CIN, 1], F32, tag="bo0", name="bo0")
                nc.vector.tensor_tensor(out=bo0[:], in0=pbw[:], in1=bo_sb[:],
                                        op=ALU.add)

                # ---- diag(scale) tiles: the transpose matmul applies the
                # per-atom LN scaling for free (rhs = diag instead of I)
                diag = {}
                for l, scl in ((2, rr[2]), (0, rstd), (1, rr[1])):
                    dg = workp.tile([NR, NR], BF16, tag=f"diag{l}", name=f"diag{l}")
                    nc.gpsimd.tensor_scalar_mul(dg[:], ident[0:NR, 0:NR],
                                                scl[:, 0:1])
                    diag[l] = dg
                dnm = workp.tile([NR, NR], BF16, tag="dnm", name="dnm")
                nc.gpsimd.tensor_scalar_mul(dnm[:], ident[0:NR, 0:NR], nmr[:, 0:1])

                # ---- per s (l=2 block first): scale+transpose -> lnT -> yT
                y_sb = workp.tile([CIN, S * NR], F32, tag="ysb", name="ysb")
                out_dma = [nc.sync, nc.scalar]
                for si, s in enumerate([4, 5, 6, 7, 8, 0, 1, 2, 3]):
                    l = int(L_OF_M[s])
                    pl = plbp.tile([128, 2 * NR], F32, tag="plb", name="plb")
                    for c in range(2):
                        nc.tensor.matmul(
                            pl[:, c * NR:(c + 1) * NR],
                            lhsT=lnin[:, s * CH + c * 128: s * CH + (c + 1) * 128],
                            rhs=diag[l][:],
                            start=True, stop=(s > 0),
                            skip_group_check=True,
                        )
                        if s == 0:
                            nc.tensor.matmul(
                                pl[:, c * NR:(c + 1) * NR],
                                lhsT=ones_r[:],
                                rhs=dnm[:],
                                start=False, stop=True,
                                skip_group_check=True,
                            )
                    lnT = tpp.tile([128, 2 * NR], BF16, tag="lnT", name="lnT")
                    if si % 2:
                        nc.vector.tensor_copy(out=lnT[:], in_=pl[:])
                    else:
                        nc.scalar.copy(lnT[:], pl[:])
                    py = pygp.tile([CIN, NR], F32, tag="pyg", name="pyg")
                    for c in range(2):
                        nc.tensor.matmul(
                            py[:],
                            lhsT=woe_g[c][:, l * CIN:(l + 1) * CIN],
                            rhs=lnT[:, c * NR:(c + 1) * NR],
                            start=(c == 0), stop=(c == 1),
                        )
                    bias = bo0[:, 0:1] if s == 0 else bo_sb[:, 0:1]
                    if si % 2:
                        nc.vector.tensor_scalar_add(y_sb[:, s * NR:(s + 1) * NR],
                                                    py[:], bias)
                    else:
                        nc.scalar.activation(y_sb[:, s * NR:(s + 1) * NR], py[:],
                                             AF.Identity, bias=bias)
                    if s == 3:
                        nc.sync.dma_start(y_d[:], y_sb[:])

    _split_multiwaits(nc)
    return nc


# ------------------------------------------------------------------ host side
def _prep_inputs(inputs: dict[str, np.ndarray]):
    """Split the full inputs into per-core in_maps for the two phases
    (index bookkeeping and value re-layout only; all arithmetic on device)."""
    q = np.asarray(inputs["q"], np.float32).reshape(N, S, CIN)
    k = np.asarray(inputs["k"], np.float32).reshape(N, S, CIN)
    v = np.asarray(inputs["v"], np.float32).reshape(N, S, CIN)
    # host pre-transpose to [i, (s, m)] and cast to bf16; vT mt-major
    qT = np.ascontiguousarray(q.transpose(2, 1, 0).reshape(CIN, S * N)).astype(NP_BF16)
    kT = np.ascontiguousarray(k.transpose(2, 1, 0).reshape(CIN, S * N)).astype(NP_BF16)
    vT = np.ascontiguousarray(
        v.reshape(NT, 128, S, CIN).transpose(3, 0, 2, 1).reshape(CIN, S * N)
    ).astype(NP_BF16)
    env = np.asarray(inputs["envelope"], np.float32)
    attn_bias = np.asarray(inputs["attn_bias"], np.float32)
    a_idx = np.asarray(inputs["atom_index"]).astype(np.int64)
    b_idx = np.asarray(inputs["batch_index"]).astype(np.int64)
    e_map = np.asarray(inputs["edge_map_tab"]).astype(np.int64)
    Wq = np.asarray(inputs["Wq"], np.float32)
    Wk = np.asarray(inputs["Wk"], np.float32)
    Wv = np.asarray(inputs["Wv"], np.float32)
    bq = np.asarray(inputs["bq"], np.float32)
    bk = np.asarray(inputs["bk"], np.float32)
    bv = np.asarray(inputs["bv"], np.float32)
    gamma = np.asarray(inputs["gamma"], np.float32)
    beta = np.asarray(inputs["beta"], np.float32)
    Wo = np.asarray(inputs["Wo"], np.float32)
    bo = np.asarray(inputs["bo"], np.float32)

    # ---- slot layout for the (atom, segment) cells
    cell = a_idx * NSEG + b_idx                      # [E]
    order = np.argsort(cell, kind="stable")
    cell_s = cell[order]
    counts = np.bincount(cell_s, minlength=N * NSEG)
    L2 = int(counts.max())
    starts = np.zeros(N * NSEG, np.int64)
    starts[1:] = np.cumsum(counts)[:-1]
    rank = np.arange(E) - starts[cell_s]             # rank within cell
    m_s = cell_s // NSEG
    g_s = cell_s % NSEG
    p_s = m_s % 128
    t_s = m_s // 128
    col = (t_s * NSEG + g_s) * L2 + rank             # free-dim position
    Wd = NT * NSEG * L2
    env_e = env[e_map]                               # value gather (re-layout)
    envS = np.zeros((128, Wd), np.float32)
    envS[p_s, col] = env_e[order]
    bS_all = []
    for h in range(H):
        bs = np.zeros((128, Wd), np.float32)
        bs[p_s, col] = attn_bias[h, e_map][order]
        bS_all.append(bs)

    # ---- per-head weight slices, expanded per spherical component, [i,(s,o)]
    WqE = Wq[L_OF_M]                                 # [9, CIN, CH]
    WkE = Wk[L_OF_M]
    WvE = Wv[L_OF_M]

    in_maps1 = []
    for h in range(H):
        sl = slice(h * D, (h + 1) * D)
        in_maps1.append({
            "qT": qT, "kT": kT, "vT": vT,
            "wq": np.ascontiguousarray(
                WqE[:, :, sl].transpose(1, 0, 2).reshape(CIN, S * D)).astype(NP_BF16),
            "wk": np.ascontiguousarray(
                WkE[:, :, sl].transpose(1, 0, 2).reshape(CIN, S * D)).astype(NP_BF16),
            "wv": np.ascontiguousarray(
                WvE[:, :, sl].transpose(1, 0, 2).reshape(CIN, S * D)).astype(NP_BF16),
            "bqkv": np.ascontiguousarray(
                np.stack([bq[sl], bk[sl], bv[sl]], axis=1)
            ),
            "bvrow": np.ascontiguousarray(bv[sl].reshape(1, D)),
            "envs": envS,
            "bs": bS_all[h],
        })

    # ---- phase-2 constants
    # gcol[p, 2l+c] = gamma[l, c*128+p];  bcol[p, c] = beta[c*128+p]
    gcol = np.zeros((128, 2 * (LMAX + 1)), np.float32)
    for l in range(LMAX + 1):
        for c in range(2):
            gcol[:, 2 * l + c] = gamma[l, c * 128:(c + 1) * 128]
    bcol = np.stack([beta[0:128], beta[128:256]], axis=1).astype(NP_BF16)
    woe = np.zeros((2, 128, (LMAX + 1) * CIN), NP_BF16)
    for c in range(2):
        woe[c] = Wo[:, c * 128:(c + 1) * 128, :].transpose(1, 0, 2).reshape(
            128, (LMAX + 1) * CIN).astype(NP_BF16)
    p2_const = {"gcol": gcol, "bcol": bcol, "woe": woe,
                "bo": np.ascontiguousarray(bo.reshape(CIN, 1))}
    return in_maps1, L2, p2_const


def _reorder_ao(ao_all: list[np.ndarray]) -> list[np.ndarray]:
    """[h][N, (s,d)] -> per-core [64, (s, h*D+d)] slices (pure data movement)."""
    full = np.stack([np.asarray(a).reshape(N, S, D) for a in ao_all], axis=2)
    full = full.reshape(N, S * CH)                                # [N, (S, H*D)]
    return [np.ascontiguousarray(full[c * NR:(c + 1) * NR]).astype(NP_BF16)
            for c in range(H)]


_BUILD_CACHE: dict = {}


def kernel(**inputs) -> np.ndarray:
    in_maps1, L2, p2_const = _prep_inputs(inputs)
    nc1 = _BUILD_CACHE.get(("p1", L2))
    if nc1 is None:
        nc1 = build_bass(L2)
        _BUILD_CACHE[("p1", L2)] = nc1
    res1 = run_bass_kernel_spmd(nc1, in_maps1, core_ids=list(range(H)))
    lnin_slices = _reorder_ao([r["ao"] for r in res1.results])

    nc2 = _BUILD_CACHE.get("p2")
    if nc2 is None:
        nc2 = build_phase2()
        _BUILD_CACHE["p2"] = nc2
    in_maps2 = [{"lnin": lnin_slices[c], **p2_const} for c in range(H)]
    res2 = run_bass_kernel_spmd(nc2, in_maps2, core_ids=list(range(H)))
    # yT [ci, (s, n_local)] per core -> y [N, S, CIN]
    y = np.zeros((N, S, CIN), np.float32)
    for c in range(H):
        yt = res2.results[c]["yT"].reshape(CIN, S, NR)
        y[c * NR:(c + 1) * NR] = yt.transpose(2, 1, 0)
    return np.ascontiguousarray(y)
